# revision 1
# baseline (speedup 1.0000x reference)
"""GCN 2-layer encoder on 8 TRN2 NeuronCores (Bass/Tile).

Math (PyG GCNConv, symmetric normalization, self-loops, deg from dst):
    out1 = relu(Dh @ A @ Dh @ (x @ W1) + b1),  Dh = diag(deg^-1/2)
    out  = Dh @ A @ Dh @ (relu1 @ W2) + b2

Factorization used here (per layer):
    table = Dh @ (feat @ W)          # per-node rows, built on device
    agg[d] = sum_{e: src->d} table[src]   (self loops included as edges)
    out[d] = dinv[d] * agg[d] + b

Sharding: nodes are assigned to 8 cores (balanced by in-degree). Each core
aggregates only its own dst nodes. Aggregation is a sigma-matrix (multi-hot
lane->column) matmul accumulating in PSUM: edges of each dst are packed into
one or more SBUF "lanes"; gathered message chunks [128 lanes, F] are
multiplied by a per-tile constant sigma [128 lanes, 128 cols] on the PE.

Messages are fetched with the SWDGE dma_gather instruction (int16 indices).
Since indices are int16, the node table is split in two blocks (cores 0-3 /
cores 4-7) and each (tile, block) run is a separate gather call.

Layer-1 tables are built replicated on every core; the layer-2 table is
built sharded and exchanged with one AllGather.
"""

import sys
import types

sys.path.insert(0, "/opt/trn_rl_repo")

import numpy as np

# Register the NTFF profile hook the container's antenv stub lacks, so
# BASS_TRACE=1 profiling works under axon (harmless otherwise).
if "antenv.axon_hooks" not in sys.modules:
    try:
        from trn_agent_boot.trn_boot import _ntff_profile_via_ctypes

        _hook = _ntff_profile_via_ctypes("/opt/axon/libaxon_pjrt.so")
    except Exception:
        _hook = None
    _m = types.ModuleType("antenv.axon_hooks")
    _m.get_axon_ntff_profile_hook = lambda: _hook
    sys.modules["antenv.axon_hooks"] = _m

N = 50000
E = 800000
IN_CH = 128
HID = 128
OUT_CH = 64
NCORES = 8
P = 128
CAP = 12  # max edges per lane per block-side
GSZ = 4  # tiles per gather call group
CALL_CAP = 8  # max chunks (x128 idxs) per dma_gather call; larger calls fail on HW
SWDGE_QUEUES = 1  # SWDGE queues to spread gather desc-gen over

_CACHE = {}
LAST_RESULTS = None


# ----------------------------------------------------------------------------
# Host-side planning
# ----------------------------------------------------------------------------
def _plan(edge_index):
    src = np.asarray(edge_index[0], dtype=np.int64)
    dst = np.asarray(edge_index[1], dtype=np.int64)
    loops = np.arange(N, dtype=np.int64)
    src_all = np.concatenate([src, loops])
    dst_all = np.concatenate([dst, loops])
    deg = np.bincount(dst_all, minlength=N)
    dinv = (1.0 / np.sqrt(deg.astype(np.float64))).astype(np.float32)

    # --- node -> core (snake over degree-sorted nodes: balances sum(deg)) ---
    order = np.argsort(-deg, kind="stable")
    snake = np.tile(
        np.concatenate([np.arange(NCORES), np.arange(NCORES - 1, -1, -1)]),
        N // (2 * NCORES) + 1,
    )[:N]
    core_of = np.empty(N, dtype=np.int64)
    core_of[order] = snake

    # --- per-dst A/B in-edge counts (A = src on cores 0-3) ------------------
    isA = core_of[src_all] < (NCORES // 2)
    a_cnt = np.bincount(dst_all[isA], minlength=N)
    b_cnt = np.bincount(dst_all[~isA], minlength=N)

    # --- per-core lane packing ---------------------------------------------
    n_lanes = np.maximum(
        1, np.maximum(-(-a_cnt // CAP), -(-b_cnt // CAP))
    ).astype(np.int64)

    # pack each core's nodes into tiles of <=128 lanes, heavy lanes first
    core_tiles = []  # per core: list of tiles; tile = list of node ids
    for c in range(NCORES):
        nodes = np.where(core_of == c)[0]
        la = -(-a_cnt[nodes] // n_lanes[nodes])
        lb = -(-b_cnt[nodes] // n_lanes[nodes])
        o2 = np.argsort(-(la + lb), kind="stable")
        tiles = []
        cur = []
        cur_lanes = 0
        for i in o2:
            nd = nodes[i]
            nl = n_lanes[nd]
            if cur_lanes + nl > P:
                tiles.append(cur)
                cur = []
                cur_lanes = 0
            cur.append(nd)
            cur_lanes += nl
        if cur:
            tiles.append(cur)
        core_tiles.append(tiles)

    # per-core per-tile chunk needs
    def tile_needs(tile_nodes):
        if not tile_nodes:
            return 0, 0
        nds = np.asarray(tile_nodes)
        ca = int(np.max(-(-a_cnt[nds] // n_lanes[nds])))
        cb = int(np.max(-(-b_cnt[nds] // n_lanes[nds])))
        return ca, cb

    needs = []  # [core][tile] = (ca, cb)
    for c in range(NCORES):
        ns = [tile_needs(t) for t in core_tiles[c]]
        # sort tiles by total need desc (keeps node lists aligned)
        o3 = sorted(range(len(ns)), key=lambda i: -(ns[i][0] + ns[i][1]))
        core_tiles[c] = [core_tiles[c][i] for i in o3]
        needs.append([ns[i] for i in o3])

    # global tile count: +1 guarantees an empty last tile on every core
    # (its column 127 is the guaranteed zero row used for gather padding)
    T = max(len(t) for t in core_tiles) + 1
    SLOTS = T * P
    assert (NCORES // 2) * SLOTS <= 32768, (T, SLOTS)

    CA = np.zeros(T, dtype=np.int64)
    CB = np.zeros(T, dtype=np.int64)
    for c in range(NCORES):
        for p_, (ca, cb) in enumerate(needs[c]):
            CA[p_] = max(CA[p_], ca)
            CB[p_] = max(CB[p_], cb)
    # every tile gets at least one chunk so PSUM is always initialized
    zero = (CA + CB) == 0
    CA[zero] = 1

    # --- slot assignment ----------------------------------------------------
    slot_of = np.full(N, -1, dtype=np.int64)  # slot within core
    lane0_of = np.full(N, -1, dtype=np.int64)  # first lane within tile
    col_of = np.full(N, -1, dtype=np.int64)
    tile_of = np.full(N, -1, dtype=np.int64)
    for c in range(NCORES):
        for p_, tile_nodes in enumerate(core_tiles[c]):
            lane = 0
            for col, nd in enumerate(tile_nodes):
                tile_of[nd] = p_
                col_of[nd] = col
                lane0_of[nd] = lane
                slot_of[nd] = p_ * P + col
                lane += n_lanes[nd]
            assert lane <= P
    pos_of = core_of * SLOTS + slot_of  # global table position

    # --- CSR of edges grouped by (dst, side) -------------------------------
    side = (~isA).astype(np.int64)
    eorder = np.argsort(dst_all * 2 + side, kind="stable")
    src_pos_sorted = pos_of[src_all[eorder]].astype(np.int64)
    estart = np.zeros(N + 1, dtype=np.int64)
    np.cumsum(deg, out=estart[1:])

    # --- gather index arrays + sigma ---------------------------------------
    G = -(-T // GSZ)
    groups = [list(range(g * GSZ, min((g + 1) * GSZ, T))) for g in range(G)]
    PAD = SLOTS - 1
    HALF = (NCORES // 2) * SLOTS

    tot_chunks = int(np.sum(CA) + np.sum(CB))
    idx_cores = []
    sigma_cores = []
    dinv_own_cores = []
    for c in range(NCORES):
        tiles = core_tiles[c]
        blocksA = [np.full((int(CA[p_]), P), PAD, np.int64) for p_ in range(T)]
        blocksB = [np.full((int(CB[p_]), P), PAD, np.int64) for p_ in range(T)]
        sig = np.zeros((T, P, P), dtype=np.float16)
        dvo = np.zeros((P, T), dtype=np.float32)
        for p_ in range(min(len(tiles), T)):
            for nd in tiles[p_]:
                nl = int(n_lanes[nd])
                l0 = int(lane0_of[nd])
                col = int(col_of[nd])
                sig[p_, l0 : l0 + nl, col] = 1.0
                dvo[col, p_] = dinv[nd]
                s0 = int(estart[nd])
                a = int(a_cnt[nd])
                b = int(b_cnt[nd])
                asrc = src_pos_sorted[s0 : s0 + a]
                bsrc = src_pos_sorted[s0 + a : s0 + a + b] - HALF
                for j in range(nl):
                    ach = asrc[j::nl]
                    bch = bsrc[j::nl]
                    if len(ach):
                        blocksA[p_][: len(ach), l0 + j] = ach
                    if len(bch):
                        blocksB[p_][: len(bch), l0 + j] = bch
        flat = []
        for g in groups:
            for p_ in g:
                flat.append(blocksA[p_].reshape(-1))
            for p_ in g:
                flat.append(blocksB[p_].reshape(-1))
        flat = np.concatenate(flat) if flat else np.zeros(0, np.int64)
        assert flat.size == tot_chunks * P
        assert flat.min() >= 0 and flat.max() < HALF
        wrapped = flat.astype(np.int16).reshape(-1, 16).T.copy()  # [16, n/16]
        idx_cores.append(np.tile(wrapped, (8, 1)))  # replicate to 128 parts
        sigma_cores.append(sig)
        dinv_own_cores.append(dvo)

    # dinv for the whole table (all cores' slots), [128, 8*T]
    dinv_all = np.zeros((P, NCORES * T), dtype=np.float32)
    for c in range(NCORES):
        dinv_all[:, c * T : (c + 1) * T] = dinv_own_cores[c]

    return dict(
        T=T,
        SLOTS=SLOTS,
        CA=CA,
        CB=CB,
        groups=groups,
        tot_chunks=tot_chunks,
        core_of=core_of,
        slot_of=slot_of,
        pos_of=pos_of,
        dinv=dinv,
        idx_cores=idx_cores,
        sigma_cores=sigma_cores,
        dinv_own_cores=dinv_own_cores,
        dinv_all=dinv_all,
    )


# ----------------------------------------------------------------------------
# Device kernel
# ----------------------------------------------------------------------------
def _build(T, CA, CB, groups, tot_chunks, use_gather=True, use_collective=True):
    import concourse.bass as bass
    import concourse.mybir as mybir
    import concourse.tile as tile
    from concourse import bacc

    f16 = mybir.dt.float16
    f32 = mybir.dt.float32
    i16 = mybir.dt.int16
    SLOTS = T * P
    ROWS = NCORES * SLOTS
    HALFROWS = ROWS // 2
    NT = NCORES * T
    max_ca = max(int(sum(CA[p_] for p_ in g)) for g in groups)
    max_cb = max(int(sum(CB[p_] for p_ in g)) for g in groups)

    nc = bacc.Bacc(
        "TRN2",
        target_bir_lowering=False,
        num_devices=NCORES,
        num_swdge_queues=SWDGE_QUEUES,
    )
    qn = [0]

    def _next_q():
        qn[0] = (qn[0] + 1) % SWDGE_QUEUES
        return qn[0]

    xT_in = nc.dram_tensor("xT", [NT, P, P], f16, kind="ExternalInput")
    w1_in = nc.dram_tensor("W1", [IN_CH, HID], f16, kind="ExternalInput")
    w2_in = nc.dram_tensor("W2", [HID, OUT_CH], f16, kind="ExternalInput")
    b1_in = nc.dram_tensor("b1bc", [P, HID], f32, kind="ExternalInput")
    b2_in = nc.dram_tensor("b2bc", [P, OUT_CH], f32, kind="ExternalInput")
    id_in = nc.dram_tensor("ident", [P, P], f16, kind="ExternalInput")
    sig_in = nc.dram_tensor("sigma", [T, P, P], f16, kind="ExternalInput")
    da_in = nc.dram_tensor("dinv_all", [P, NT], f32, kind="ExternalInput")
    do_in = nc.dram_tensor("dinv_own", [P, T], f32, kind="ExternalInput")
    idx_in = nc.dram_tensor("idx", [P, tot_chunks * 8], i16, kind="ExternalInput")
    out_ext = nc.dram_tensor("out", [SLOTS, OUT_CH], f32, kind="ExternalOutput")

    with tile.TileContext(nc) as tc:
        with (
            tc.tile_pool(name="const", bufs=1) as cpool,
            tc.tile_pool(name="xt", bufs=3) as xtpool,
            tc.tile_pool(name="sig", bufs=3) as sigpool,
            tc.tile_pool(name="stg", bufs=2) as stgpool,
            tc.tile_pool(name="drain", bufs=3) as dpool,
            tc.tile_pool(name="psb", bufs=2, space="PSUM") as ps_build,
            tc.tile_pool(name="psa", bufs=2, space="PSUM") as ps_agg,
            tc.tile_pool(name="pst", bufs=2, space="PSUM") as ps_tr,
            tc.tile_pool(name="psm", bufs=2, space="PSUM") as ps_mm2,
            tc.tile_pool(name="dram", bufs=1, space="DRAM") as dram,
        ):
            # ---- constants into SBUF ----
            w1_sb = cpool.tile([IN_CH, HID], f16)
            nc.sync.dma_start(out=w1_sb[:], in_=w1_in[:])
            w2_sb = cpool.tile([HID, OUT_CH], f16)
            nc.sync.dma_start(out=w2_sb[:], in_=w2_in[:])
            b1_sb = cpool.tile([P, HID], f32)
            nc.sync.dma_start(out=b1_sb[:], in_=b1_in[:])
            b2_sb = cpool.tile([P, OUT_CH], f32)
            nc.sync.dma_start(out=b2_sb[:], in_=b2_in[:])
            id_sb = cpool.tile([P, P], f16)
            nc.sync.dma_start(out=id_sb[:], in_=id_in[:])
            da_sb = cpool.tile([P, NT], f32)
            nc.sync.dma_start(out=da_sb[:], in_=da_in[:])
            do_sb = cpool.tile([P, T], f32)
            nc.sync.dma_start(out=do_sb[:], in_=do_in[:])
            idx_sb = cpool.tile([P, tot_chunks * 8], i16)
            nc.sync.dma_start(out=idx_sb[:], in_=idx_in[:])

            table1 = dram.tile([ROWS, HID], f16)
            shard2 = dram.tile([SLOTS, P], f16)
            table2 = dram.tile([ROWS, P], f16, addr_space="Shared" if use_collective else "Local")

            # ---- phase 1: table1 = dinv * (x @ W1), full, replicated ----
            for j in range(NT):
                xt_t = xtpool.tile([P, P], f16, tag="xt")
                nc.sync.dma_start(out=xt_t[:], in_=xT_in[j])
                bps = ps_build.tile([P, HID], f32, tag="build")
                nc.tensor.matmul(
                    bps[:], lhsT=xt_t[:], rhs=w1_sb[:], start=True, stop=True
                )
                h1t = xtpool.tile([P, HID], f16, tag="h1t")
                if j % 2 == 0:
                    nc.scalar.activation(
                        h1t[:],
                        bps[:],
                        mybir.ActivationFunctionType.Copy,
                        scale=da_sb[:, j : j + 1],
                    )
                else:
                    nc.vector.tensor_scalar_mul(h1t[:], bps[:], da_sb[:, j : j + 1])
                nc.sync.dma_start(out=table1[j * P : (j + 1) * P, :], in_=h1t[:])

            # ---- per-layer aggregation ----
            def aggregate(layer):
                tab = table1 if layer == 0 else table2
                nfeat = HID if layer == 0 else OUT_CH
                coff = 0
                for g in groups:
                    ca_g = int(sum(int(CA[p_]) for p_ in g))
                    cb_g = int(sum(int(CB[p_]) for p_ in g))
                    stA = stB = None
                    if ca_g:
                        stA = stgpool.tile([P, max_ca, P], f16, tag="stgA")
                        if use_gather:
                            for s_ in range(0, ca_g, CALL_CAP):
                                n_ = min(CALL_CAP, ca_g - s_)
                                nc.gpsimd.dma_gather(
                                    stA[:, s_ : s_ + n_, :],
                                    tab[0:HALFROWS, :],
                                    idx_sb[:, (coff + s_) * 8 : (coff + s_ + n_) * 8],
                                    n_ * P,
                                    n_ * P,
                                    P,
                                    queue_num=_next_q(),
                                )
                        else:
                            nc.sync.dma_start(
                                out=stA[:, 0:ca_g, :],
                                in_=tab[0 : ca_g * P, :].rearrange(
                                    "(c p) f -> p c f", p=P
                                ),
                            )
                    if cb_g:
                        stB = stgpool.tile([P, max_cb, P], f16, tag="stgB")
                        if use_gather:
                            for s_ in range(0, cb_g, CALL_CAP):
                                n_ = min(CALL_CAP, cb_g - s_)
                                nc.gpsimd.dma_gather(
                                    stB[:, s_ : s_ + n_, :],
                                    tab[HALFROWS:ROWS, :],
                                    idx_sb[
                                        :,
                                        (coff + ca_g + s_) * 8 : (coff + ca_g + s_ + n_) * 8,
                                    ],
                                    n_ * P,
                                    n_ * P,
                                    P,
                                    queue_num=_next_q(),
                                )
                        else:
                            nc.sync.dma_start(
                                out=stB[:, 0:cb_g, :],
                                in_=tab[0 : cb_g * P, :].rearrange(
                                    "(c p) f -> p c f", p=P
                                ),
                            )
                    a_off = 0
                    b_off = 0
                    for p_ in g:
                        sg = sigpool.tile([P, P], f16, tag="sig")
                        nc.sync.dma_start(out=sg[:], in_=sig_in[p_])
                        aps = ps_agg.tile([P, nfeat], f32, tag="agg")
                        ntot = int(CA[p_]) + int(CB[p_])
                        k = 0
                        for ci in range(int(CA[p_])):
                            nc.tensor.matmul(
                                aps[:],
                                lhsT=sg[:],
                                rhs=stA[:, a_off + ci, 0:nfeat],
                                start=(k == 0),
                                stop=(k == ntot - 1),
                            )
                            k += 1
                        for ci in range(int(CB[p_])):
                            nc.tensor.matmul(
                                aps[:],
                                lhsT=sg[:],
                                rhs=stB[:, b_off + ci, 0:nfeat],
                                start=(k == 0),
                                stop=(k == ntot - 1),
                            )
                            k += 1
                        a_off += int(CA[p_])
                        b_off += int(CB[p_])
                        drain(layer, p_, aps)
                    coff += ca_g + cb_g

            def drain(layer, p_, aps):
                dv = do_sb[:, p_ : p_ + 1]
                if layer == 0:
                    # r1 = dinv*agg + b1 ; r3 = relu(r1)*dinv (fp16)
                    r1 = dpool.tile([P, HID], f32, tag="r1")
                    nc.scalar.activation(
                        r1[:], aps[:], mybir.ActivationFunctionType.Copy, scale=dv
                    )
                    nc.vector.tensor_add(r1[:], r1[:], b1_sb[:])
                    r3 = dpool.tile([P, HID], f16, tag="r3")
                    nc.vector.tensor_scalar(
                        r3[:], r1[:], 0.0, dv, mybir.AluOpType.max, mybir.AluOpType.mult
                    )
                    psT = ps_tr.tile([P, P], f16, tag="tr")
                    nc.tensor.transpose(psT[:], r3[:], id_sb[:])
                    rT = dpool.tile([P, P], f16, tag="rT")
                    nc.vector.tensor_copy(rT[:], psT[:])
                    ps2 = ps_mm2.tile([P, OUT_CH], f32, tag="mm2")
                    nc.tensor.matmul(
                        ps2[:], lhsT=rT[:], rhs=w2_sb[:], start=True, stop=True
                    )
                    t2 = dpool.tile([P, P], f16, tag="t2")
                    nc.scalar.activation(
                        t2[:, 0:OUT_CH], ps2[:], mybir.ActivationFunctionType.Copy
                    )
                    nc.vector.memset(t2[:, OUT_CH:P], 0.0)
                    nc.sync.dma_start(
                        out=shard2[p_ * P : (p_ + 1) * P, :], in_=t2[:]
                    )
                else:
                    o1 = dpool.tile([P, OUT_CH], f32, tag="o1")
                    nc.scalar.activation(
                        o1[:], aps[:], mybir.ActivationFunctionType.Copy, scale=dv
                    )
                    nc.vector.tensor_add(o1[:], o1[:], b2_sb[:])
                    nc.sync.dma_start(
                        out=out_ext[p_ * P : (p_ + 1) * P, :], in_=o1[:]
                    )

            aggregate(0)

            if use_collective:
                nc.gpsimd.collective_compute(
                    "AllGather",
                    mybir.AluOpType.bypass,
                    replica_groups=[list(range(NCORES))],
                    ins=[shard2.opt()],
                    outs=[table2.opt()],
                )
            else:
                for c_ in range(NCORES):
                    nc.sync.dma_start(
                        out=table2[c_ * SLOTS : (c_ + 1) * SLOTS, :], in_=shard2[:]
                    )

            aggregate(1)

    nc.compile()  # bacc passes: library loads, register allocation, DCE
    _split_sync_waits(nc, mybir, max_waits=1)
    return nc


def _split_sync_waits(nc, mybir, max_waits=1):
    """This walrus build rejects instructions with more than `max_waits` sync
    waits; hoist excess waits onto injected same-engine InstNoOps."""
    n_split = 0
    for fn in nc.m.functions:
        for bb in fn.blocks:
            out = []
            changed = False
            for ins in bb.instructions:
                si = ins.sync_info
                if si is not None and si.on_wait and len(si.on_wait) > max_waits:
                    waits = list(si.on_wait)
                    excess = waits[:-max_waits]
                    for i in range(0, len(excess), max_waits):
                        nop = mybir.InstNoOp(
                            name=nc.get_next_instruction_name(),
                            sync_info=mybir.SyncInfo(
                                on_wait=excess[i : i + max_waits], on_update=[]
                            ),
                            bass_nofuse=True,
                            engine=ins.engine,
                        )
                        out.append(nop)
                        n_split += 1
                    si.on_wait = waits[-max_waits:]
                    ins.sync_info = si
                    changed = True
                out.append(ins)
            if changed:
                bb.instructions = out
    return n_split


# ----------------------------------------------------------------------------
# Entry point
# ----------------------------------------------------------------------------
def kernel(x, edge_index, W1, b1, W2, b2):
    global LAST_RESULTS
    from concourse.bass_utils import run_bass_kernel_spmd

    x = np.asarray(x)
    W1a = np.asarray(W1)
    b1a = np.asarray(b1)
    W2a = np.asarray(W2)
    b2a = np.asarray(b2)

    key = hash(np.asarray(edge_index)[:, :: E // 997].tobytes())
    if key not in _CACHE:
        plan = _plan(edge_index)
        nc = _build(
            plan["T"], plan["CA"], plan["CB"], plan["groups"], plan["tot_chunks"]
        )
        _CACHE[key] = (plan, nc)
    plan, nc = _CACHE[key]

    T = plan["T"]
    SLOTS = plan["SLOTS"]
    NT = NCORES * T

    # xT in table order, tile-major: [NT, 128 infeat, 128 nodes]
    xT = np.zeros((NT, P, P), dtype=np.float16)
    nodes = np.arange(N)
    gpos = plan["pos_of"]  # global table position per node
    xTflat = np.zeros((P, NCORES * SLOTS), dtype=np.float16)
    xTflat[:, gpos] = x.astype(np.float16).T
    xT[:] = xTflat.reshape(P, NT, P).transpose(1, 0, 2)

    in_common = {
        "xT": xT,
        "W1": W1a.astype(np.float16),
        "W2": W2a.astype(np.float16),
        "b1bc": np.broadcast_to(b1a.astype(np.float32), (P, HID)).copy(),
        "b2bc": np.broadcast_to(b2a.astype(np.float32), (P, OUT_CH)).copy(),
        "ident": np.eye(P, dtype=np.float16),
        "dinv_all": plan["dinv_all"],
    }
    in_maps = []
    for c in range(NCORES):
        m = dict(in_common)
        m["sigma"] = plan["sigma_cores"][c]
        m["dinv_own"] = plan["dinv_own_cores"][c]
        m["idx"] = plan["idx_cores"][c]
        in_maps.append(m)

    res = run_bass_kernel_spmd(nc, in_maps, core_ids=list(range(NCORES)))
    LAST_RESULTS = res

    out = np.empty((N, OUT_CH), dtype=np.float32)
    core_of = plan["core_of"]
    slot_of = plan["slot_of"]
    for c in range(NCORES):
        sel = core_of == c
        out[sel] = res.results[c]["out"][slot_of[sel]]
    return out



# revision 16
# speedup vs baseline: 1.1804x; 1.1804x over previous
"""GCN 2-layer encoder on 8 TRN2 NeuronCores (Bass/Tile).

Math (PyG GCNConv, symmetric normalization, self-loops, deg from dst):
    out1 = relu(Dh @ A @ Dh @ (x @ W1) + b1),  Dh = diag(deg^-1/2)
    out  = Dh @ A @ Dh @ (relu1 @ W2) + b2

Factorization used here (per layer):
    table = Dh @ (feat @ W)          # per-node rows, built on device
    agg[d] = sum_{e: src->d} table[src]   (self loops included as edges)
    out[d] = dinv[d] * agg[d] + b

Sharding: nodes are assigned to 8 cores (balanced by in-degree); each core
aggregates its own dst nodes. Per dst tile (128 nodes), in-edges are packed
densely into chunks of 128 lanes; gathered message chunks [128 lanes, F]
are multiplied on the PE by a per-chunk multi-hot sigma (lane -> dst col)
accumulating in PSUM. Sigma matrices are built on-device by the Vector
engine (iota == colidx compare) from compact per-lane column indices, so
lanes need no static lane->node binding and padding is just the final
partial chunk per (tile, side): ~5% vs ~56% for per-tile sigma.

Messages are fetched with SWDGE dma_gather in prepare_only mode + explicit
trigger_dma: desc-gen (~0.7us/call) is decoupled from the DMA transfer
(~5-8us/call), which otherwise blocks the GpSimd engine. Post-compile
surgery (_fix_swdge_prep_sems) wires the descriptor completion sems to the
DMASW lane sems Tile's consumers actually wait on, throttles to one
in-flight call per lane, and restores the dropped write-after-read hazard
(trigger vs. previous staging-slot readers).

Since gather indices are int16, the node table is split in two blocks
(cores 0-3 / cores 4-7); each (tile-group, block) run is a separate call.

Layer-1 tables are built replicated on every core; the layer-2 table is
built sharded and exchanged with one AllGather.
"""

import sys
import types

sys.path.insert(0, "/opt/trn_rl_repo")

import numpy as np

# Register the NTFF profile hook the container's antenv stub lacks, so
# BASS_TRACE=1 profiling works under axon (harmless otherwise).
if "antenv.axon_hooks" not in sys.modules:
    try:
        from trn_agent_boot.trn_boot import _ntff_profile_via_ctypes

        _hook = _ntff_profile_via_ctypes("/opt/axon/libaxon_pjrt.so")
    except Exception:
        _hook = None
    _m = types.ModuleType("antenv.axon_hooks")
    _m.get_axon_ntff_profile_hook = lambda: _hook
    sys.modules["antenv.axon_hooks"] = _m

N = 50000
E = 800000
IN_CH = 128
HID = 128
OUT_CH = 64
NCORES = 8
P = 128
GSZ = 4  # tiles per gather call group
CALL_CAP = 8  # max chunks (x128 idxs) per dma_gather call (16KB/engine packet)
SWDGE_QUEUES = 4
BB = 4  # phase-1 DMA batching (tiles per dma_start)

_CACHE = {}
LAST_RESULTS = None


# ----------------------------------------------------------------------------
# Host-side planning
# ----------------------------------------------------------------------------
def _plan(edge_index):
    src = np.asarray(edge_index[0], dtype=np.int64)
    dst = np.asarray(edge_index[1], dtype=np.int64)
    loops = np.arange(N, dtype=np.int64)
    src_all = np.concatenate([src, loops])
    dst_all = np.concatenate([dst, loops])
    deg = np.bincount(dst_all, minlength=N)
    dinv = (1.0 / np.sqrt(deg.astype(np.float64))).astype(np.float32)

    # node -> core: snake over degree-sorted nodes (balances sum(deg))
    order = np.argsort(-deg, kind="stable")
    snake = np.tile(
        np.concatenate([np.arange(NCORES), np.arange(NCORES - 1, -1, -1)]),
        N // (2 * NCORES) + 1,
    )[:N]
    core_of = np.empty(N, dtype=np.int64)
    core_of[order] = snake

    isA = core_of[src_all] < (NCORES // 2)
    a_cnt = np.bincount(dst_all[isA], minlength=N)
    b_cnt = np.bincount(dst_all[~isA], minlength=N)

    # node -> (tile, col): per core, snake over degree-sorted nodes across
    # provisional tiles (balances per-tile edge sums), tiles then sorted by
    # chunk need desc (aligns profiles across cores) and renumbered.
    tile_of = np.full(N, -1, dtype=np.int64)
    col_of = np.full(N, -1, dtype=np.int64)
    ntiles_max = 0
    prov = []
    for c in range(NCORES):
        nodes = np.where(core_of == c)[0]
        nn = len(nodes)
        ntiles = -(-nn // P)
        ntiles_max = max(ntiles_max, ntiles)
        o2 = np.argsort(-(a_cnt[nodes] + b_cnt[nodes]), kind="stable")
        nds = nodes[o2]
        sn = np.tile(
            np.concatenate([np.arange(ntiles), np.arange(ntiles - 1, -1, -1)]),
            nn // (2 * ntiles) + 1,
        )[:nn]
        prov.append([nds[sn == t] for t in range(ntiles)])

    T = ntiles_max
    ca_t = np.zeros((NCORES, T), dtype=np.int64)
    cb_t = np.zeros((NCORES, T), dtype=np.int64)
    for c in range(NCORES):
        for t, nds in enumerate(prov[c]):
            ca_t[c, t] = -(-int(a_cnt[nds].sum()) // P)
            cb_t[c, t] = -(-int(b_cnt[nds].sum()) // P)
    CA = np.zeros(T, dtype=np.int64)
    CB = np.zeros(T, dtype=np.int64)
    for c in range(NCORES):
        perm = sorted(
            range(len(prov[c])), key=lambda t: -(ca_t[c, t] + cb_t[c, t])
        )
        for p_, t in enumerate(perm):
            nds = prov[c][t]
            tile_of[nds] = p_
            col_of[nds] = np.arange(len(nds))
            CA[p_] = max(CA[p_], ca_t[c, t])
            CB[p_] = max(CB[p_], cb_t[c, t])
    CA[(CA + CB) == 0] = 1

    SLOTS = T * P
    HALFROWS = (NCORES // 2) * SLOTS
    assert HALFROWS <= 32768, HALFROWS
    slot_of = tile_of * P + col_of
    pos_of = core_of * SLOTS + slot_of

    # pad rows: any unoccupied slot is a zero row in both tables (zero x,
    # dinv=0). Find one in core 3 (A half) and core 7 (B half).
    def free_slot(c):
        occ = np.zeros(SLOTS, dtype=bool)
        occ[slot_of[core_of == c]] = True
        fr = np.where(~occ)[0]
        assert len(fr) > 0
        return int(fr[-1])

    PAD_A = (NCORES // 2 - 1) * SLOTS + free_slot(NCORES // 2 - 1)
    PAD_B = (NCORES // 2 - 1) * SLOTS + free_slot(NCORES - 1)

    ecore = core_of[dst_all]
    etile = tile_of[dst_all]
    eside = (~isA).astype(np.int64)
    esrcpos = pos_of[src_all]
    ecol = col_of[dst_all]

    G = -(-T // GSZ)
    groups = [list(range(g * GSZ, min((g + 1) * GSZ, T))) for g in range(G)]
    tot_chunks = int(np.sum(CA) + np.sum(CB))
    maxc_call = 0
    for g in groups:
        ca_g = int(sum(CA[p_] for p_ in g))
        cb_g = int(sum(CB[p_] for p_ in g))
        maxc_call = max(maxc_call, ca_g, cb_g)

    ekey = np.lexsort((esrcpos, etile, eside, ecore))
    es_core = ecore[ekey]
    es_side = eside[ekey]
    es_tile = etile[ekey]
    es_srcpos = esrcpos[ekey]
    es_col = ecol[ekey]
    keyv = (es_core * 2 + es_side) * T + es_tile
    uniq, starts = np.unique(keyv, return_index=True)
    ends = np.append(starts[1:], len(keyv))
    bnd = {int(u): (int(s0), int(e0)) for u, s0, e0 in zip(uniq, starts, ends)}

    idx_cores = []
    colidx_cores = []
    dinv_own_cores = []
    for c in range(NCORES):
        flat_idx = []
        flat_col = []
        dvo = np.zeros((P, T), dtype=np.float32)
        nds_c = np.where(core_of == c)[0]
        dvo[col_of[nds_c], tile_of[nds_c]] = dinv[nds_c]

        def emit(side, t, nchunks):
            k = (c * 2 + side) * T + t
            s0, e0 = bnd.get(k, (0, 0))
            sp = es_srcpos[s0:e0]
            cl = es_col[s0:e0]
            if side == 1:
                sp = sp - HALFROWS
            n_ = e0 - s0
            want = nchunks * P
            ii = np.full(want, PAD_A if side == 0 else PAD_B, np.int64)
            cc = np.full(want, P - 1, np.int64)
            ii[:n_] = sp
            cc[:n_] = cl
            flat_idx.append(ii)
            flat_col.append(cc)

        for g in groups:
            for p_ in g:
                emit(0, p_, int(CA[p_]))
            for p_ in g:
                emit(1, p_, int(CB[p_]))
        fi = np.concatenate(flat_idx)
        fc = np.concatenate(flat_col)
        assert fi.size == tot_chunks * P
        assert fi.min() >= 0 and fi.max() < HALFROWS
        wrapped = fi.astype(np.int16).reshape(-1, 16).T.copy()
        idx_cores.append(np.tile(wrapped, (8, 1)))
        colidx_cores.append(fc.reshape(tot_chunks, P).T.astype(np.float16).copy())
        dinv_own_cores.append(dvo)

    dinv_all = np.zeros((P, NCORES * T), dtype=np.float32)
    for c in range(NCORES):
        dinv_all[:, c * T : (c + 1) * T] = dinv_own_cores[c]

    iotaC = np.tile(
        np.arange(P, dtype=np.float16)[None, :], (P, maxc_call)
    ).reshape(P, maxc_call * P)

    return dict(
        T=T,
        SLOTS=SLOTS,
        CA=CA,
        CB=CB,
        groups=groups,
        tot_chunks=tot_chunks,
        maxc_call=maxc_call,
        core_of=core_of,
        slot_of=slot_of,
        pos_of=pos_of,
        dinv=dinv,
        idx_cores=idx_cores,
        colidx_cores=colidx_cores,
        dinv_own_cores=dinv_own_cores,
        dinv_all=dinv_all,
        iotaC=iotaC,
    )


# ----------------------------------------------------------------------------
# Device kernel
# ----------------------------------------------------------------------------
def _build(
    T,
    CA,
    CB,
    groups,
    tot_chunks,
    maxc_call,
    use_collective=True,
    detect_races=True,
):
    import concourse.bass as bass
    import concourse.mybir as mybir
    import concourse.tile as tile
    from concourse import bacc

    f16 = mybir.dt.float16
    f32 = mybir.dt.float32
    i16 = mybir.dt.int16
    SLOTS = T * P
    ROWS = NCORES * SLOTS
    HALFROWS = ROWS // 2
    NT = NCORES * T

    nc = bacc.Bacc(
        "TRN2",
        target_bir_lowering=False,
        num_devices=NCORES,
        num_swdge_queues=SWDGE_QUEUES,
        detect_race_conditions=detect_races,
    )
    qn = [0]

    def _next_q():
        qn[0] = (qn[0] + 1) % SWDGE_QUEUES
        return qn[0]

    dma_sems = [nc.alloc_semaphore(f"swdge_dma_q{i}") for i in range(SWDGE_QUEUES)]

    def _gather(out_ap, in_ap, idx_ap, n_idx):
        q = _next_q()
        nc.gpsimd.dma_gather(
            out_ap,
            in_ap,
            idx_ap,
            n_idx,
            n_idx,
            P,
            prepare_only=True,
            sem=dma_sems[q],
            queue_num=q,
        )
        nc.gpsimd.trigger_dma(count=None, queue_num=q)

    xT_in = nc.dram_tensor("xT", [NT, P, P], f16, kind="ExternalInput")
    w1_in = nc.dram_tensor("W1", [IN_CH, HID], f16, kind="ExternalInput")
    w2_in = nc.dram_tensor("W2", [HID, OUT_CH], f16, kind="ExternalInput")
    b1_in = nc.dram_tensor("b1bc", [P, HID], f32, kind="ExternalInput")
    b2_in = nc.dram_tensor("b2bc", [P, OUT_CH], f32, kind="ExternalInput")
    id_in = nc.dram_tensor("ident", [P, P], f16, kind="ExternalInput")
    col_in = nc.dram_tensor("colidx", [P, tot_chunks], f16, kind="ExternalInput")
    iota_in = nc.dram_tensor("iotaC", [P, maxc_call * P], f16, kind="ExternalInput")
    da_in = nc.dram_tensor("dinv_all", [P, NT], f32, kind="ExternalInput")
    do_in = nc.dram_tensor("dinv_own", [P, T], f32, kind="ExternalInput")
    idx_in = nc.dram_tensor("idx", [P, tot_chunks * 8], i16, kind="ExternalInput")
    out_ext = nc.dram_tensor("out", [SLOTS, OUT_CH], f32, kind="ExternalOutput")

    with tile.TileContext(nc) as tc:
        with (
            tc.tile_pool(name="const", bufs=1) as cpool,
            tc.tile_pool(name="xt", bufs=3) as xtpool,
            tc.tile_pool(name="sig", bufs=2) as sigpool,
            tc.tile_pool(name="stg", bufs=2) as stgpool,
            tc.tile_pool(name="drain", bufs=3) as dpool,
            tc.tile_pool(name="psb", bufs=2, space="PSUM") as ps_build,
            tc.tile_pool(name="psa", bufs=2, space="PSUM") as ps_agg,
            tc.tile_pool(name="pst", bufs=2, space="PSUM") as ps_tr,
            tc.tile_pool(name="psm", bufs=2, space="PSUM") as ps_mm2,
            tc.tile_pool(name="dram", bufs=1, space="DRAM") as dram,
        ):
            # ---- constants into SBUF ----
            w1_sb = cpool.tile([IN_CH, HID], f16)
            nc.sync.dma_start(out=w1_sb[:], in_=w1_in[:])
            w2_sb = cpool.tile([HID, OUT_CH], f16)
            nc.sync.dma_start(out=w2_sb[:], in_=w2_in[:])
            b1_sb = cpool.tile([P, HID], f32)
            nc.sync.dma_start(out=b1_sb[:], in_=b1_in[:])
            b2_sb = cpool.tile([P, OUT_CH], f32)
            nc.sync.dma_start(out=b2_sb[:], in_=b2_in[:])
            id_sb = cpool.tile([P, P], f16)
            nc.sync.dma_start(out=id_sb[:], in_=id_in[:])
            col_sb = cpool.tile([P, tot_chunks], f16)
            nc.sync.dma_start(out=col_sb[:], in_=col_in[:])
            iota_sb = cpool.tile([P, maxc_call * P], f16)
            nc.sync.dma_start(out=iota_sb[:], in_=iota_in[:])
            da_sb = cpool.tile([P, NT], f32)
            nc.sync.dma_start(out=da_sb[:], in_=da_in[:])
            do_sb = cpool.tile([P, T], f32)
            nc.sync.dma_start(out=do_sb[:], in_=do_in[:])
            idx_sb = cpool.tile([P, tot_chunks * 8], i16)
            nc.sync.dma_start(out=idx_sb[:], in_=idx_in[:])

            table1 = dram.tile([ROWS, HID], f16)
            shard2 = dram.tile([SLOTS, P], f16)
            table2 = dram.tile(
                [ROWS, P], f16, addr_space="Shared" if use_collective else "Local"
            )

            # ---- phase 1: table1 = dinv * (x @ W1), full, replicated ----
            for j0 in range(0, NT, BB):
                nb = min(BB, NT - j0)
                xt_t = xtpool.tile([P, nb * P], f16, tag="xt")
                nc.sync.dma_start(
                    out=xt_t[:].rearrange("p (t c) -> p t c", t=nb),
                    in_=xT_in[j0 : j0 + nb].rearrange("t p c -> p t c"),
                )
                h1t = xtpool.tile([P, nb * HID], f16, tag="h1t")
                for k in range(nb):
                    j = j0 + k
                    bps = ps_build.tile([P, HID], f32, tag="build")
                    nc.tensor.matmul(
                        bps[:],
                        lhsT=xt_t[:, k * P : (k + 1) * P],
                        rhs=w1_sb[:],
                        start=True,
                        stop=True,
                    )
                    if j % 2 == 0:
                        nc.scalar.activation(
                            h1t[:, k * HID : (k + 1) * HID],
                            bps[:],
                            mybir.ActivationFunctionType.Copy,
                            scale=da_sb[:, j : j + 1],
                        )
                    else:
                        nc.vector.tensor_scalar_mul(
                            h1t[:, k * HID : (k + 1) * HID],
                            bps[:],
                            da_sb[:, j : j + 1],
                        )
                nc.sync.dma_start(
                    out=table1[j0 * P : (j0 + nb) * P, :].rearrange(
                        "(t p) f -> p t f", t=nb
                    ),
                    in_=h1t[:].rearrange("p (t f) -> p t f", t=nb),
                )

            # ---- per-layer aggregation ----
            def aggregate(layer):
                tab = table1 if layer == 0 else table2
                nfeat = HID if layer == 0 else OUT_CH
                coff = 0
                for g in groups:
                    ca_g = int(sum(int(CA[p_]) for p_ in g))
                    cb_g = int(sum(int(CB[p_]) for p_ in g))
                    stA = stB = sgA = sgB = None
                    if ca_g:
                        stA = stgpool.tile([P, maxc_call, P], f16, tag="stgA")
                        for s_ in range(0, ca_g, CALL_CAP):
                            n_ = min(CALL_CAP, ca_g - s_)
                            _gather(
                                stA[:, s_ : s_ + n_, :],
                                tab[0:HALFROWS, :],
                                idx_sb[:, (coff + s_) * 8 : (coff + s_ + n_) * 8],
                                n_ * P,
                            )
                        sgA = sigpool.tile([P, maxc_call * P], f16, tag="sgA")
                        nc.vector.tensor_tensor(
                            sgA[:, : ca_g * P].rearrange(
                                "p (k c) -> p k c", k=ca_g
                            ),
                            iota_sb[:, : ca_g * P].rearrange(
                                "p (k c) -> p k c", k=ca_g
                            ),
                            col_sb[:, coff : coff + ca_g]
                            .unsqueeze(-1)
                            .broadcast_to([P, ca_g, P]),
                            mybir.AluOpType.is_equal,
                        )
                    if cb_g:
                        stB = stgpool.tile([P, maxc_call, P], f16, tag="stgB")
                        for s_ in range(0, cb_g, CALL_CAP):
                            n_ = min(CALL_CAP, cb_g - s_)
                            _gather(
                                stB[:, s_ : s_ + n_, :],
                                tab[HALFROWS:ROWS, :],
                                idx_sb[
                                    :,
                                    (coff + ca_g + s_) * 8 : (coff + ca_g + s_ + n_)
                                    * 8,
                                ],
                                n_ * P,
                            )
                        sgB = sigpool.tile([P, maxc_call * P], f16, tag="sgB")
                        nc.vector.tensor_tensor(
                            sgB[:, : cb_g * P].rearrange(
                                "p (k c) -> p k c", k=cb_g
                            ),
                            iota_sb[:, : cb_g * P].rearrange(
                                "p (k c) -> p k c", k=cb_g
                            ),
                            col_sb[:, coff + ca_g : coff + ca_g + cb_g]
                            .unsqueeze(-1)
                            .broadcast_to([P, cb_g, P]),
                            mybir.AluOpType.is_equal,
                        )
                    a_off = 0
                    b_off = 0
                    for p_ in g:
                        aps = ps_agg.tile([P, nfeat], f32, tag="agg")
                        ntot = int(CA[p_]) + int(CB[p_])
                        k = 0
                        for ci in range(int(CA[p_])):
                            cc = a_off + ci
                            nc.tensor.matmul(
                                aps[:],
                                lhsT=sgA[:, cc * P : (cc + 1) * P],
                                rhs=stA[:, cc, 0:nfeat],
                                start=(k == 0),
                                stop=(k == ntot - 1),
                            )
                            k += 1
                        for ci in range(int(CB[p_])):
                            cc = b_off + ci
                            nc.tensor.matmul(
                                aps[:],
                                lhsT=sgB[:, cc * P : (cc + 1) * P],
                                rhs=stB[:, cc, 0:nfeat],
                                start=(k == 0),
                                stop=(k == ntot - 1),
                            )
                            k += 1
                        a_off += int(CA[p_])
                        b_off += int(CB[p_])
                        drain(layer, p_, aps)
                    coff += ca_g + cb_g

            def drain(layer, p_, aps):
                dv = do_sb[:, p_ : p_ + 1]
                if layer == 0:
                    # r1 = dinv*agg + b1 ; r3 = relu(r1)*dinv (fp16)
                    r1 = dpool.tile([P, HID], f32, tag="r1")
                    nc.scalar.activation(
                        r1[:], aps[:], mybir.ActivationFunctionType.Copy, scale=dv
                    )
                    nc.vector.tensor_add(r1[:], r1[:], b1_sb[:])
                    r3 = dpool.tile([P, HID], f16, tag="r3")
                    nc.vector.tensor_scalar(
                        r3[:], r1[:], 0.0, dv, mybir.AluOpType.max, mybir.AluOpType.mult
                    )
                    psT = ps_tr.tile([P, P], f16, tag="tr")
                    nc.tensor.transpose(psT[:], r3[:], id_sb[:])
                    rT = dpool.tile([P, P], f16, tag="rT")
                    nc.vector.tensor_copy(rT[:], psT[:])
                    ps2 = ps_mm2.tile([P, OUT_CH], f32, tag="mm2")
                    nc.tensor.matmul(
                        ps2[:], lhsT=rT[:], rhs=w2_sb[:], start=True, stop=True
                    )
                    t2 = dpool.tile([P, P], f16, tag="t2")
                    nc.scalar.activation(
                        t2[:, 0:OUT_CH], ps2[:], mybir.ActivationFunctionType.Copy
                    )
                    nc.vector.memset(t2[:, OUT_CH:P], 0.0)
                    nc.sync.dma_start(
                        out=shard2[p_ * P : (p_ + 1) * P, :], in_=t2[:]
                    )
                else:
                    o1 = dpool.tile([P, OUT_CH], f32, tag="o1")
                    nc.scalar.activation(
                        o1[:], aps[:], mybir.ActivationFunctionType.Copy, scale=dv
                    )
                    nc.vector.tensor_add(o1[:], o1[:], b2_sb[:])
                    nc.sync.dma_start(
                        out=out_ext[p_ * P : (p_ + 1) * P, :], in_=o1[:]
                    )

            aggregate(0)

            if use_collective:
                nc.gpsimd.collective_compute(
                    "AllGather",
                    mybir.AluOpType.bypass,
                    replica_groups=[list(range(NCORES))],
                    ins=[shard2.opt()],
                    outs=[table2.opt()],
                )
            else:
                for c_ in range(NCORES):
                    nc.sync.dma_start(
                        out=table2[c_ * SLOTS : (c_ + 1) * SLOTS, :], in_=shard2[:]
                    )

            aggregate(1)

    nc.compile()  # bacc passes: library loads, register allocation, DCE
    _fix_swdge_prep_sems(nc, mybir)
    _split_sync_waits(nc, mybir, max_waits=1)
    return nc


PREP_DEPTH = 2  # max in-flight gather calls per SWDGE queue


def _fix_swdge_prep_sems(nc, mybir):
    """Post-compile surgery for the gen_mode==1 SWDGE prep/trigger path.

    Tile treats prepare_only SWDGE completion as user-managed: it
    discharges the DMASW lane ticks with unconditional IncSwdgeSem
    pre-bumps, so the lane-sem waits it emits on consumers are vacuous.
    The author must enforce data readiness with the sem= semaphores
    (one per queue here, descriptors bump +16 per call). Enforce:

    1. Data RAW: the first matmul reading each staging-tile instance
       waits on every covering gather call: sem_q >= 16*(call# in q + 1).
    2. WAR: the trigger that fires a DMA overwriting a staging slot waits
       on PE engine sem >= (last matmul reading the slot's previous
       instance; staging pools have bufs=2, so that is the same-tag
       instance two allocations back).
    3. Ring/throttle: prep #j on queue q waits sem_q >= 16*(j-D+1),
       capping in-flight calls per queue at D=PREP_DEPTH.
    """
    import re

    queue_sems = {}
    pe_sem = None
    for fn in nc.m.functions:
        for bb in fn.blocks:
            for ins in bb.instructions:
                si = ins.sync_info
                if not si:
                    continue
                for u in si.on_update or []:
                    nm = u.ant_name or ""
                    if nm.startswith("swdge_dma_q"):
                        queue_sems[int(nm[11:])] = (u.id, nm)
                for w in si.on_wait or []:
                    nm = w.ant_name or ""
                    if nm.startswith("PE_") and pe_sem is None:
                        pe_sem = (w.id, nm)

    pat = re.compile(r"\b(st[AB])_(\d+)\b")

    def _stg_name(ap):
        m = pat.search(str(ap))
        return m.group(0) if m else None

    streams = []
    for fn in nc.m.functions:
        for bb in fn.blocks:
            streams.append(bb.instructions)

    # pass 1: per-queue call indices per prep; staging instances: creation
    # order (per tag), covering calls, first/last matmul readers
    inst_order = {"stA": [], "stB": []}
    seen = set()
    first_reader = {}
    last_reader_n = {}
    inst_calls = {}
    prep_info = {}
    q_count = {}
    pe_n = 0
    for insts in streams:
        for ins in insts:
            tn = type(ins).__name__
            if tn == "InstMatmult":
                pe_n += 1
                for ap in ins.ins or []:
                    nm = _stg_name(ap)
                    if nm:
                        if nm not in first_reader:
                            first_reader[nm] = ins
                        last_reader_n[nm] = pe_n
            elif tn == "InstDMAGatherAnt" and getattr(ins, "gen_mode", 0) == 1:
                q = ins.queue_num
                jq = q_count.get(q, 0)
                q_count[q] = jq + 1
                nm = _stg_name(ins.outs[0])
                prep_info[ins.name] = (q, jq, nm)
                if nm:
                    inst_calls.setdefault(nm, []).append((q, jq))
                    if nm not in seen:
                        seen.add(nm)
                        inst_order[nm[:3]].append(nm)
    prev_inst = {}
    for tag, lst in inst_order.items():
        for i, nm in enumerate(lst):
            if i >= 2:
                prev_inst[nm] = lst[i - 2]

    def _add_wait(ins, sid, snm, val):
        si = ins.sync_info
        if si is None:
            si = mybir.SyncInfo(on_wait=[], on_update=[])
        si.on_wait = list(si.on_wait or []) + [
            mybir.SyncWait(
                sync_type="semaphore",
                id=sid,
                ant_name=snm,
                wait_mode="sem-ge-imm",
                wait_value=val,
                wait_reg=None,
            )
        ]
        ins.sync_info = si

    # 1. data RAW waits on first readers
    for nm, rd in first_reader.items():
        per_q = {}
        for q, jq in inst_calls.get(nm, []):
            per_q[q] = max(per_q.get(q, -1), jq)
        for q, jq in sorted(per_q.items()):
            if q in queue_sems:
                sid, snm = queue_sems[q]
                _add_wait(rd, sid, snm, 16 * (jq + 1))

    # 2 + 3. WAR waits on triggers, throttle on preps; also gate the first
    # trigger after each collective on its completion (the gather source
    # table2 is written by the async AllGather).
    cc_sem = None
    for insts in streams:
        for ins in insts:
            si = ins.sync_info
            if not si:
                continue
            for w in si.on_wait or []:
                if (w.ant_name or "").startswith("Collectives"):
                    cc_sem = (w.id, w.ant_name)
    pending_prep = {}
    cc_count = 0
    cc_pending = False
    for insts in streams:
        for ins in insts:
            tn = type(ins).__name__
            if tn == "InstCollectiveCompute":
                cc_count += 1
                cc_pending = True
            elif tn == "InstDMAGatherAnt" and getattr(ins, "gen_mode", 0) == 1:
                q, jq, nm = prep_info[ins.name]
                pending_prep[q] = ins.name
                if jq >= PREP_DEPTH and q in queue_sems:
                    sid, snm = queue_sems[q]
                    _add_wait(ins, sid, snm, 16 * (jq - PREP_DEPTH + 1))
            elif tn == "InstTriggerDma":
                if cc_pending and cc_sem is not None:
                    _add_wait(ins, cc_sem[0], cc_sem[1], cc_count)
                    cc_pending = False
                pn = pending_prep.pop(ins.queue_num, None)
                if pn is None or pe_sem is None:
                    continue
                nm = prep_info[pn][2]
                prev = prev_inst.get(nm) if nm else None
                tgt = last_reader_n.get(prev, 0) if prev else 0
                if tgt > 0:
                    sid, snm = pe_sem
                    _add_wait(ins, sid, snm, tgt)


def _split_sync_waits(nc, mybir, max_waits=1):
    """This walrus build rejects instructions with more than `max_waits` sync
    waits; hoist excess waits onto injected same-engine InstNoOps."""
    n_split = 0
    for fn in nc.m.functions:
        for bb in fn.blocks:
            out = []
            changed = False
            for ins in bb.instructions:
                si = ins.sync_info
                if si is not None and si.on_wait and len(si.on_wait) > max_waits:
                    waits = list(si.on_wait)
                    excess = waits[:-max_waits]
                    for i in range(0, len(excess), max_waits):
                        nop = mybir.InstNoOp(
                            name=nc.get_next_instruction_name(),
                            sync_info=mybir.SyncInfo(
                                on_wait=excess[i : i + max_waits], on_update=[]
                            ),
                            bass_nofuse=True,
                            engine=ins.engine,
                        )
                        out.append(nop)
                        n_split += 1
                    si.on_wait = waits[-max_waits:]
                    ins.sync_info = si
                    changed = True
                out.append(ins)
            if changed:
                bb.instructions = out
    return n_split


# ----------------------------------------------------------------------------
# Entry point
# ----------------------------------------------------------------------------
def kernel(x, edge_index, W1, b1, W2, b2):
    global LAST_RESULTS
    from concourse.bass_utils import run_bass_kernel_spmd

    x = np.asarray(x)
    W1a = np.asarray(W1)
    b1a = np.asarray(b1)
    W2a = np.asarray(W2)
    b2a = np.asarray(b2)

    key = hash(np.asarray(edge_index)[:, :: E // 997].tobytes())
    if key not in _CACHE:
        plan = _plan(edge_index)
        nc = _build(
            plan["T"],
            plan["CA"],
            plan["CB"],
            plan["groups"],
            plan["tot_chunks"],
            plan["maxc_call"],
        )
        _CACHE[key] = (plan, nc)
    plan, nc = _CACHE[key]

    T = plan["T"]
    SLOTS = plan["SLOTS"]
    NT = NCORES * T

    # xT in table order, tile-major: [NT, 128 infeat, 128 nodes]
    xT = np.zeros((NT, P, P), dtype=np.float16)
    gpos = plan["pos_of"]
    xTflat = np.zeros((P, NCORES * SLOTS), dtype=np.float16)
    xTflat[:, gpos] = x.astype(np.float16).T
    xT[:] = xTflat.reshape(P, NT, P).transpose(1, 0, 2)

    in_common = {
        "xT": xT,
        "W1": W1a.astype(np.float16),
        "W2": W2a.astype(np.float16),
        "b1bc": np.broadcast_to(b1a.astype(np.float32), (P, HID)).copy(),
        "b2bc": np.broadcast_to(b2a.astype(np.float32), (P, OUT_CH)).copy(),
        "ident": np.eye(P, dtype=np.float16),
        "dinv_all": plan["dinv_all"],
        "iotaC": plan["iotaC"],
    }
    in_maps = []
    for c in range(NCORES):
        m = dict(in_common)
        m["colidx"] = plan["colidx_cores"][c]
        m["dinv_own"] = plan["dinv_own_cores"][c]
        m["idx"] = plan["idx_cores"][c]
        in_maps.append(m)

    res = run_bass_kernel_spmd(nc, in_maps, core_ids=list(range(NCORES)))
    LAST_RESULTS = res

    out = np.empty((N, OUT_CH), dtype=np.float32)
    core_of = plan["core_of"]
    slot_of = plan["slot_of"]
    for c in range(NCORES):
        sel = core_of == c
        out[sel] = res.results[c]["out"][slot_of[sel]]
    return out


# revision 22
# speedup vs baseline: 1.3504x; 1.1440x over previous
"""GCN 2-layer encoder on 8 TRN2 NeuronCores (Bass/Tile).

Math (PyG GCNConv, symmetric normalization, self-loops, deg from dst):
    out1 = relu(Dh @ A @ Dh @ (x @ W1) + b1),  Dh = diag(deg^-1/2)
    out  = Dh @ A @ Dh @ (relu1 @ W2) + b2

Factorization used here (per layer):
    table = Dh @ (feat @ W)          # per-node rows, built on device
    agg[d] = sum_{e: src->d} table[src]   (self loops included as edges)
    out[d] = dinv[d] * agg[d] + b

Sharding: nodes are assigned to 8 cores (balanced by in-degree); each core
aggregates its own dst nodes. Per dst tile (128 nodes), in-edges are packed
densely into chunks of 128 lanes; gathered message chunks [128 lanes, F]
are multiplied on the PE by a per-chunk multi-hot sigma (lane -> dst col)
accumulating in PSUM. Sigma matrices are built on-device by the Vector
engine (iota == colidx compare) from compact per-lane column indices, so
lanes need no static lane->node binding and padding is just the final
partial chunk per (tile, side): ~5% vs ~56% for per-tile sigma.

Messages are fetched with SWDGE dma_gather in prepare_only mode + explicit
trigger_dma: desc-gen (~0.7us/call) is decoupled from the DMA transfer
(~5-8us/call), which otherwise blocks the GpSimd engine. Post-compile
surgery (_fix_swdge_prep_sems) wires the descriptor completion sems to the
DMASW lane sems Tile's consumers actually wait on, throttles to one
in-flight call per lane, and restores the dropped write-after-read hazard
(trigger vs. previous staging-slot readers).

Since gather indices are int16, the node table is split in two blocks
(cores 0-3 / cores 4-7); each (tile-group, block) run is a separate call.

Layer-1 tables are built replicated on every core; the layer-2 table is
built sharded and exchanged with one AllGather.
"""

import sys
import types

sys.path.insert(0, "/opt/trn_rl_repo")

import numpy as np

# Register the NTFF profile hook the container's antenv stub lacks, so
# BASS_TRACE=1 profiling works under axon (harmless otherwise).
if "antenv.axon_hooks" not in sys.modules:
    try:
        from trn_agent_boot.trn_boot import _ntff_profile_via_ctypes

        _hook = _ntff_profile_via_ctypes("/opt/axon/libaxon_pjrt.so")
    except Exception:
        _hook = None
    _m = types.ModuleType("antenv.axon_hooks")
    _m.get_axon_ntff_profile_hook = lambda: _hook
    sys.modules["antenv.axon_hooks"] = _m

N = 50000
E = 800000
IN_CH = 128
HID = 128
OUT_CH = 64
NCORES = 8
P = 128
GSZ = 4  # tiles per gather call group
CALL_CAP = 8  # max chunks (x128 idxs) per dma_gather call (16KB/engine packet)
SWDGE_QUEUES = 4
BB = 4  # phase-1 DMA batching (tiles per dma_start)

_CACHE = {}
LAST_RESULTS = None


# ----------------------------------------------------------------------------
# Host-side planning
# ----------------------------------------------------------------------------
def _plan(edge_index):
    src = np.asarray(edge_index[0], dtype=np.int64)
    dst = np.asarray(edge_index[1], dtype=np.int64)
    loops = np.arange(N, dtype=np.int64)
    src_all = np.concatenate([src, loops])
    dst_all = np.concatenate([dst, loops])
    deg = np.bincount(dst_all, minlength=N)
    dinv = (1.0 / np.sqrt(deg.astype(np.float64))).astype(np.float32)

    # node -> core: snake over degree-sorted nodes (balances sum(deg))
    order = np.argsort(-deg, kind="stable")
    snake = np.tile(
        np.concatenate([np.arange(NCORES), np.arange(NCORES - 1, -1, -1)]),
        N // (2 * NCORES) + 1,
    )[:N]
    core_of = np.empty(N, dtype=np.int64)
    core_of[order] = snake

    isA = core_of[src_all] < (NCORES // 2)
    a_cnt = np.bincount(dst_all[isA], minlength=N)
    b_cnt = np.bincount(dst_all[~isA], minlength=N)

    # node -> (tile, col): per core, snake over degree-sorted nodes across
    # provisional tiles (balances per-tile edge sums), tiles then sorted by
    # chunk need desc (aligns profiles across cores) and renumbered.
    tile_of = np.full(N, -1, dtype=np.int64)
    col_of = np.full(N, -1, dtype=np.int64)
    ntiles_max = 0
    prov = []
    for c in range(NCORES):
        nodes = np.where(core_of == c)[0]
        nn = len(nodes)
        ntiles = -(-nn // P)
        ntiles_max = max(ntiles_max, ntiles)
        o2 = np.argsort(-(a_cnt[nodes] + b_cnt[nodes]), kind="stable")
        nds = nodes[o2]
        sn = np.tile(
            np.concatenate([np.arange(ntiles), np.arange(ntiles - 1, -1, -1)]),
            nn // (2 * ntiles) + 1,
        )[:nn]
        prov.append([nds[sn == t] for t in range(ntiles)])

    T = ntiles_max
    ca_t = np.zeros((NCORES, T), dtype=np.int64)
    cb_t = np.zeros((NCORES, T), dtype=np.int64)
    for c in range(NCORES):
        for t, nds in enumerate(prov[c]):
            ca_t[c, t] = -(-int(a_cnt[nds].sum()) // P)
            cb_t[c, t] = -(-int(b_cnt[nds].sum()) // P)
    CA = np.zeros(T, dtype=np.int64)
    CB = np.zeros(T, dtype=np.int64)
    for c in range(NCORES):
        perm = sorted(
            range(len(prov[c])), key=lambda t: -(ca_t[c, t] + cb_t[c, t])
        )
        for p_, t in enumerate(perm):
            nds = prov[c][t]
            tile_of[nds] = p_
            col_of[nds] = np.arange(len(nds))
            CA[p_] = max(CA[p_], ca_t[c, t])
            CB[p_] = max(CB[p_], cb_t[c, t])
    CA[(CA + CB) == 0] = 1

    SLOTS = T * P
    HALFROWS = (NCORES // 2) * SLOTS
    assert HALFROWS <= 32768, HALFROWS
    slot_of = tile_of * P + col_of
    pos_of = core_of * SLOTS + slot_of

    # pad rows: any unoccupied slot is a zero row in both tables (zero x,
    # dinv=0). Find one in core 3 (A half) and core 7 (B half).
    def free_slot(c):
        occ = np.zeros(SLOTS, dtype=bool)
        occ[slot_of[core_of == c]] = True
        fr = np.where(~occ)[0]
        assert len(fr) > 0
        return int(fr[-1])

    PAD_A = (NCORES // 2 - 1) * SLOTS + free_slot(NCORES // 2 - 1)
    PAD_B = (NCORES // 2 - 1) * SLOTS + free_slot(NCORES - 1)

    ecore = core_of[dst_all]
    etile = tile_of[dst_all]
    eside = (~isA).astype(np.int64)
    esrcpos = pos_of[src_all]
    ecol = col_of[dst_all]

    G = -(-T // GSZ)
    groups = [list(range(g * GSZ, min((g + 1) * GSZ, T))) for g in range(G)]
    tot_chunks = int(np.sum(CA) + np.sum(CB))
    maxc_call = 0
    for g in groups:
        ca_g = int(sum(CA[p_] for p_ in g))
        cb_g = int(sum(CB[p_] for p_ in g))
        maxc_call = max(maxc_call, ca_g, cb_g)

    ekey = np.lexsort((esrcpos, etile, eside, ecore))
    es_core = ecore[ekey]
    es_side = eside[ekey]
    es_tile = etile[ekey]
    es_srcpos = esrcpos[ekey]
    es_col = ecol[ekey]
    keyv = (es_core * 2 + es_side) * T + es_tile
    uniq, starts = np.unique(keyv, return_index=True)
    ends = np.append(starts[1:], len(keyv))
    bnd = {int(u): (int(s0), int(e0)) for u, s0, e0 in zip(uniq, starts, ends)}

    idx_cores = []
    colidx_cores = []
    dinv_own_cores = []
    for c in range(NCORES):
        flat_idx = []
        flat_col = []
        dvo = np.zeros((P, T), dtype=np.float32)
        nds_c = np.where(core_of == c)[0]
        dvo[col_of[nds_c], tile_of[nds_c]] = dinv[nds_c]

        def emit(side, t, nchunks):
            k = (c * 2 + side) * T + t
            s0, e0 = bnd.get(k, (0, 0))
            sp = es_srcpos[s0:e0]
            cl = es_col[s0:e0]
            if side == 1:
                sp = sp - HALFROWS
            n_ = e0 - s0
            want = nchunks * P
            ii = np.full(want, PAD_A if side == 0 else PAD_B, np.int64)
            cc = np.full(want, P - 1, np.int64)
            ii[:n_] = sp
            cc[:n_] = cl
            flat_idx.append(ii)
            flat_col.append(cc)

        for g in groups:
            for p_ in g:
                emit(0, p_, int(CA[p_]))
            for p_ in g:
                emit(1, p_, int(CB[p_]))
        fi = np.concatenate(flat_idx)
        fc = np.concatenate(flat_col)
        assert fi.size == tot_chunks * P
        assert fi.min() >= 0 and fi.max() < HALFROWS
        wrapped = fi.astype(np.int16).reshape(-1, 16).T.copy()
        idx_cores.append(np.tile(wrapped, (8, 1)))
        colidx_cores.append(fc.reshape(tot_chunks, P).T.astype(np.float16).copy())
        dinv_own_cores.append(dvo)

    dinv_all = np.zeros((P, NCORES * T), dtype=np.float32)
    for c in range(NCORES):
        dinv_all[:, c * T : (c + 1) * T] = dinv_own_cores[c]

    iotaC = np.tile(
        np.arange(P, dtype=np.float16)[None, :], (P, maxc_call)
    ).reshape(P, maxc_call * P)

    return dict(
        T=T,
        SLOTS=SLOTS,
        CA=CA,
        CB=CB,
        groups=groups,
        tot_chunks=tot_chunks,
        maxc_call=maxc_call,
        core_of=core_of,
        slot_of=slot_of,
        pos_of=pos_of,
        dinv=dinv,
        idx_cores=idx_cores,
        colidx_cores=colidx_cores,
        dinv_own_cores=dinv_own_cores,
        dinv_all=dinv_all,
        iotaC=iotaC,
    )


# ----------------------------------------------------------------------------
# Device kernel
# ----------------------------------------------------------------------------
def _build(
    T,
    CA,
    CB,
    groups,
    tot_chunks,
    maxc_call,
    use_collective=True,
    detect_races=True,
):
    import concourse.bass as bass
    import concourse.mybir as mybir
    import concourse.tile as tile
    from concourse import bacc

    f16 = mybir.dt.float16
    f32 = mybir.dt.float32
    i16 = mybir.dt.int16
    SLOTS = T * P
    ROWS = NCORES * SLOTS
    HALFROWS = ROWS // 2
    NT = NCORES * T

    nc = bacc.Bacc(
        "TRN2",
        target_bir_lowering=False,
        num_devices=NCORES,
        num_swdge_queues=SWDGE_QUEUES,
        detect_race_conditions=detect_races,
    )
    qn = [0]

    def _next_q():
        qn[0] = (qn[0] + 1) % SWDGE_QUEUES
        return qn[0]

    dma_sems = [nc.alloc_semaphore(f"swdge_dma_q{i}") for i in range(SWDGE_QUEUES)]

    def _prep(out_ap, in_ap, idx_ap, n_idx, q):
        nc.gpsimd.dma_gather(
            out_ap,
            in_ap,
            idx_ap,
            n_idx,
            n_idx,
            P,
            prepare_only=True,
            sem=dma_sems[q],
            queue_num=q,
        )

    def _fire(q):
        # One trigger per (group, side): the trigger blocks the GpSimd
        # engine ~9us on HW regardless of how many calls it fires, so
        # batch all of a side's calls onto one queue and fire once.
        nc.gpsimd.trigger_dma(count=None, queue_num=q)

    xT_in = nc.dram_tensor("xT", [NT, P, P], f16, kind="ExternalInput")
    w1_in = nc.dram_tensor("W1", [IN_CH, HID], f16, kind="ExternalInput")
    w2_in = nc.dram_tensor("W2", [HID, OUT_CH], f16, kind="ExternalInput")
    b1_in = nc.dram_tensor("b1bc", [P, HID], f32, kind="ExternalInput")
    b2_in = nc.dram_tensor("b2bc", [P, OUT_CH], f32, kind="ExternalInput")
    id_in = nc.dram_tensor("ident", [P, P], f16, kind="ExternalInput")
    col_in = nc.dram_tensor("colidx", [P, tot_chunks], f16, kind="ExternalInput")
    iota_in = nc.dram_tensor("iotaC", [P, maxc_call * P], f16, kind="ExternalInput")
    da_in = nc.dram_tensor("dinv_all", [P, NT], f32, kind="ExternalInput")
    do_in = nc.dram_tensor("dinv_own", [P, T], f32, kind="ExternalInput")
    idx_in = nc.dram_tensor("idx", [P, tot_chunks * 8], i16, kind="ExternalInput")
    out_ext = nc.dram_tensor("out", [SLOTS, OUT_CH], f32, kind="ExternalOutput")

    with tile.TileContext(nc) as tc:
        with (
            tc.tile_pool(name="const", bufs=1) as cpool,
            tc.tile_pool(name="xt", bufs=3) as xtpool,
            tc.tile_pool(name="sig", bufs=2) as sigpool,
            tc.tile_pool(name="stg", bufs=2) as stgpool,
            tc.tile_pool(name="drain", bufs=3) as dpool,
            tc.tile_pool(name="psb", bufs=2, space="PSUM") as ps_build,
            tc.tile_pool(name="psa", bufs=2, space="PSUM") as ps_agg,
            tc.tile_pool(name="pst", bufs=2, space="PSUM") as ps_tr,
            tc.tile_pool(name="psm", bufs=2, space="PSUM") as ps_mm2,
            tc.tile_pool(name="dram", bufs=1, space="DRAM") as dram,
        ):
            # ---- constants into SBUF ----
            w1_sb = cpool.tile([IN_CH, HID], f16)
            nc.sync.dma_start(out=w1_sb[:], in_=w1_in[:])
            w2_sb = cpool.tile([HID, OUT_CH], f16)
            nc.sync.dma_start(out=w2_sb[:], in_=w2_in[:])
            b1_sb = cpool.tile([P, HID], f32)
            nc.sync.dma_start(out=b1_sb[:], in_=b1_in[:])
            b2_sb = cpool.tile([P, OUT_CH], f32)
            nc.sync.dma_start(out=b2_sb[:], in_=b2_in[:])
            id_sb = cpool.tile([P, P], f16)
            nc.sync.dma_start(out=id_sb[:], in_=id_in[:])
            col_sb = cpool.tile([P, tot_chunks], f16)
            nc.sync.dma_start(out=col_sb[:], in_=col_in[:])
            iota_sb = cpool.tile([P, maxc_call * P], f16)
            nc.sync.dma_start(out=iota_sb[:], in_=iota_in[:])
            da_sb = cpool.tile([P, NT], f32)
            nc.sync.dma_start(out=da_sb[:], in_=da_in[:])
            do_sb = cpool.tile([P, T], f32)
            nc.sync.dma_start(out=do_sb[:], in_=do_in[:])
            idx_sb = cpool.tile([P, tot_chunks * 8], i16)
            nc.sync.dma_start(out=idx_sb[:], in_=idx_in[:])

            table1 = dram.tile([ROWS, HID], f16)
            shard2 = dram.tile([SLOTS, P], f16)
            table2 = dram.tile(
                [ROWS, P], f16, addr_space="Shared" if use_collective else "Local"
            )

            # ---- phase 1: table1 = dinv * (x @ W1), full, replicated ----
            for j0 in range(0, NT, BB):
                nb = min(BB, NT - j0)
                xt_t = xtpool.tile([P, nb * P], f16, tag="xt")
                nc.sync.dma_start(
                    out=xt_t[:].rearrange("p (t c) -> p t c", t=nb),
                    in_=xT_in[j0 : j0 + nb].rearrange("t p c -> p t c"),
                )
                h1t = xtpool.tile([P, nb * HID], f16, tag="h1t")
                for k in range(nb):
                    j = j0 + k
                    bps = ps_build.tile([P, HID], f32, tag="build")
                    nc.tensor.matmul(
                        bps[:],
                        lhsT=xt_t[:, k * P : (k + 1) * P],
                        rhs=w1_sb[:],
                        start=True,
                        stop=True,
                    )
                    if j % 2 == 0:
                        nc.scalar.activation(
                            h1t[:, k * HID : (k + 1) * HID],
                            bps[:],
                            mybir.ActivationFunctionType.Copy,
                            scale=da_sb[:, j : j + 1],
                        )
                    else:
                        nc.vector.tensor_scalar_mul(
                            h1t[:, k * HID : (k + 1) * HID],
                            bps[:],
                            da_sb[:, j : j + 1],
                        )
                nc.sync.dma_start(
                    out=table1[j0 * P : (j0 + nb) * P, :].rearrange(
                        "(t p) f -> p t f", t=nb
                    ),
                    in_=h1t[:].rearrange("p (t f) -> p t f", t=nb),
                )

            # ---- per-layer aggregation ----
            def aggregate(layer):
                tab = table1 if layer == 0 else table2
                nfeat = HID if layer == 0 else OUT_CH
                coff = 0
                for g in groups:
                    ca_g = int(sum(int(CA[p_]) for p_ in g))
                    cb_g = int(sum(int(CB[p_]) for p_ in g))
                    stA = stB = sgA = sgB = None
                    if ca_g:
                        qa = _next_q()
                        stA = stgpool.tile([P, maxc_call, P], f16, tag="stgA")
                        for s_ in range(0, ca_g, CALL_CAP):
                            n_ = min(CALL_CAP, ca_g - s_)
                            _prep(
                                stA[:, s_ : s_ + n_, :],
                                tab[0:HALFROWS, :],
                                idx_sb[:, (coff + s_) * 8 : (coff + s_ + n_) * 8],
                                n_ * P,
                                qa,
                            )
                        _fire(qa)
                        sgA = sigpool.tile([P, maxc_call * P], f16, tag="sgA")
                        nc.vector.tensor_tensor(
                            sgA[:, : ca_g * P].rearrange(
                                "p (k c) -> p k c", k=ca_g
                            ),
                            iota_sb[:, : ca_g * P].rearrange(
                                "p (k c) -> p k c", k=ca_g
                            ),
                            col_sb[:, coff : coff + ca_g]
                            .unsqueeze(-1)
                            .broadcast_to([P, ca_g, P]),
                            mybir.AluOpType.is_equal,
                        )
                    if cb_g:
                        qb = _next_q()
                        stB = stgpool.tile([P, maxc_call, P], f16, tag="stgB")
                        for s_ in range(0, cb_g, CALL_CAP):
                            n_ = min(CALL_CAP, cb_g - s_)
                            _prep(
                                stB[:, s_ : s_ + n_, :],
                                tab[HALFROWS:ROWS, :],
                                idx_sb[
                                    :,
                                    (coff + ca_g + s_) * 8 : (coff + ca_g + s_ + n_)
                                    * 8,
                                ],
                                n_ * P,
                                qb,
                            )
                        _fire(qb)
                        sgB = sigpool.tile([P, maxc_call * P], f16, tag="sgB")
                        nc.vector.tensor_tensor(
                            sgB[:, : cb_g * P].rearrange(
                                "p (k c) -> p k c", k=cb_g
                            ),
                            iota_sb[:, : cb_g * P].rearrange(
                                "p (k c) -> p k c", k=cb_g
                            ),
                            col_sb[:, coff + ca_g : coff + ca_g + cb_g]
                            .unsqueeze(-1)
                            .broadcast_to([P, cb_g, P]),
                            mybir.AluOpType.is_equal,
                        )
                    a_off = 0
                    b_off = 0
                    for p_ in g:
                        aps = ps_agg.tile([P, nfeat], f32, tag="agg")
                        ntot = int(CA[p_]) + int(CB[p_])
                        k = 0
                        for ci in range(int(CA[p_])):
                            cc = a_off + ci
                            nc.tensor.matmul(
                                aps[:],
                                lhsT=sgA[:, cc * P : (cc + 1) * P],
                                rhs=stA[:, cc, 0:nfeat],
                                start=(k == 0),
                                stop=(k == ntot - 1),
                            )
                            k += 1
                        for ci in range(int(CB[p_])):
                            cc = b_off + ci
                            nc.tensor.matmul(
                                aps[:],
                                lhsT=sgB[:, cc * P : (cc + 1) * P],
                                rhs=stB[:, cc, 0:nfeat],
                                start=(k == 0),
                                stop=(k == ntot - 1),
                            )
                            k += 1
                        a_off += int(CA[p_])
                        b_off += int(CB[p_])
                        drain(layer, p_, aps)
                    coff += ca_g + cb_g

            def drain(layer, p_, aps):
                dv = do_sb[:, p_ : p_ + 1]
                if layer == 0:
                    # r1 = dinv*agg + b1 ; r3 = relu(r1)*dinv (fp16)
                    r1 = dpool.tile([P, HID], f32, tag="r1")
                    nc.scalar.activation(
                        r1[:], aps[:], mybir.ActivationFunctionType.Copy, scale=dv
                    )
                    nc.vector.tensor_add(r1[:], r1[:], b1_sb[:])
                    r3 = dpool.tile([P, HID], f16, tag="r3")
                    nc.vector.tensor_scalar(
                        r3[:], r1[:], 0.0, dv, mybir.AluOpType.max, mybir.AluOpType.mult
                    )
                    psT = ps_tr.tile([P, P], f16, tag="tr")
                    nc.tensor.transpose(psT[:], r3[:], id_sb[:])
                    rT = dpool.tile([P, P], f16, tag="rT")
                    nc.vector.tensor_copy(rT[:], psT[:])
                    ps2 = ps_mm2.tile([P, OUT_CH], f32, tag="mm2")
                    nc.tensor.matmul(
                        ps2[:], lhsT=rT[:], rhs=w2_sb[:], start=True, stop=True
                    )
                    t2 = dpool.tile([P, P], f16, tag="t2")
                    nc.scalar.activation(
                        t2[:, 0:OUT_CH], ps2[:], mybir.ActivationFunctionType.Copy
                    )
                    nc.vector.memset(t2[:, OUT_CH:P], 0.0)
                    nc.sync.dma_start(
                        out=shard2[p_ * P : (p_ + 1) * P, :], in_=t2[:]
                    )
                else:
                    o1 = dpool.tile([P, OUT_CH], f32, tag="o1")
                    nc.scalar.activation(
                        o1[:], aps[:], mybir.ActivationFunctionType.Copy, scale=dv
                    )
                    nc.vector.tensor_add(o1[:], o1[:], b2_sb[:])
                    nc.sync.dma_start(
                        out=out_ext[p_ * P : (p_ + 1) * P, :], in_=o1[:]
                    )

            aggregate(0)

            if use_collective:
                nc.gpsimd.collective_compute(
                    "AllGather",
                    mybir.AluOpType.bypass,
                    replica_groups=[list(range(NCORES))],
                    ins=[shard2.opt()],
                    outs=[table2.opt()],
                )
            else:
                for c_ in range(NCORES):
                    nc.sync.dma_start(
                        out=table2[c_ * SLOTS : (c_ + 1) * SLOTS, :], in_=shard2[:]
                    )

            aggregate(1)

    nc.compile()  # bacc passes: library loads, register allocation, DCE
    _fix_swdge_prep_sems(nc, mybir)
    _split_sync_waits(nc, mybir, max_waits=1)
    return nc


PREP_DEPTH = 10  # max in-flight gather calls per SWDGE queue (~2 side-batches)


def _fix_swdge_prep_sems(nc, mybir):
    """Post-compile surgery for the gen_mode==1 SWDGE prep/trigger path.

    Tile treats prepare_only SWDGE completion as user-managed: it
    discharges the DMASW lane ticks with unconditional IncSwdgeSem
    pre-bumps, so the lane-sem waits it emits on consumers are vacuous.
    The author must enforce data readiness with the sem= semaphores
    (one per queue here, descriptors bump +16 per call). Enforce:

    1. Data RAW: the first matmul reading each staging-tile instance
       waits on every covering gather call: sem_q >= 16*(call# in q + 1).
    2. WAR: the trigger that fires a DMA overwriting a staging slot waits
       on PE engine sem >= (last matmul reading the slot's previous
       instance; staging pools have bufs=2, so that is the same-tag
       instance two allocations back).
    3. Ring/throttle: prep #j on queue q waits sem_q >= 16*(j-D+1),
       capping in-flight calls per queue at D=PREP_DEPTH.
    """
    import re

    queue_sems = {}
    pe_sem = None
    for fn in nc.m.functions:
        for bb in fn.blocks:
            for ins in bb.instructions:
                si = ins.sync_info
                if not si:
                    continue
                for u in si.on_update or []:
                    nm = u.ant_name or ""
                    if nm.startswith("swdge_dma_q"):
                        queue_sems[int(nm[11:])] = (u.id, nm)
                for w in si.on_wait or []:
                    nm = w.ant_name or ""
                    if nm.startswith("PE_") and pe_sem is None:
                        pe_sem = (w.id, nm)

    pat = re.compile(r"\b(st[AB])_(\d+)\b")

    def _stg_name(ap):
        m = pat.search(str(ap))
        return m.group(0) if m else None

    streams = []
    for fn in nc.m.functions:
        for bb in fn.blocks:
            streams.append(bb.instructions)

    # pass 1: per-queue call indices per prep; staging instances: creation
    # order (per tag), covering calls, first/last matmul readers
    inst_order = {"stA": [], "stB": []}
    seen = set()
    first_reader = {}
    last_reader_n = {}
    inst_calls = {}
    prep_info = {}
    q_count = {}
    pe_n = 0
    for insts in streams:
        for ins in insts:
            tn = type(ins).__name__
            if tn == "InstMatmult":
                pe_n += 1
                for ap in ins.ins or []:
                    nm = _stg_name(ap)
                    if nm:
                        if nm not in first_reader:
                            first_reader[nm] = ins
                        last_reader_n[nm] = pe_n
            elif tn == "InstDMAGatherAnt" and getattr(ins, "gen_mode", 0) == 1:
                q = ins.queue_num
                jq = q_count.get(q, 0)
                q_count[q] = jq + 1
                nm = _stg_name(ins.outs[0])
                prep_info[ins.name] = (q, jq, nm)
                if nm:
                    inst_calls.setdefault(nm, []).append((q, jq))
                    if nm not in seen:
                        seen.add(nm)
                        inst_order[nm[:3]].append(nm)
    prev_inst = {}
    for tag, lst in inst_order.items():
        for i, nm in enumerate(lst):
            if i >= 2:
                prev_inst[nm] = lst[i - 2]

    def _add_wait(ins, sid, snm, val):
        si = ins.sync_info
        if si is None:
            si = mybir.SyncInfo(on_wait=[], on_update=[])
        si.on_wait = list(si.on_wait or []) + [
            mybir.SyncWait(
                sync_type="semaphore",
                id=sid,
                ant_name=snm,
                wait_mode="sem-ge-imm",
                wait_value=val,
                wait_reg=None,
            )
        ]
        ins.sync_info = si

    # 1. data RAW waits on first readers
    for nm, rd in first_reader.items():
        per_q = {}
        for q, jq in inst_calls.get(nm, []):
            per_q[q] = max(per_q.get(q, -1), jq)
        for q, jq in sorted(per_q.items()):
            if q in queue_sems:
                sid, snm = queue_sems[q]
                _add_wait(rd, sid, snm, 16 * (jq + 1))

    # 2 + 3. WAR waits on triggers, throttle on preps; also gate the first
    # trigger after each collective on its completion (the gather source
    # table2 is written by the async AllGather).
    cc_sem = None
    for insts in streams:
        for ins in insts:
            si = ins.sync_info
            if not si:
                continue
            for w in si.on_wait or []:
                if (w.ant_name or "").startswith("Collectives"):
                    cc_sem = (w.id, w.ant_name)
    pending_prep = {}
    cc_count = 0
    cc_pending = False
    for insts in streams:
        for ins in insts:
            tn = type(ins).__name__
            if tn == "InstCollectiveCompute":
                cc_count += 1
                cc_pending = True
            elif tn == "InstDMAGatherAnt" and getattr(ins, "gen_mode", 0) == 1:
                q, jq, nm = prep_info[ins.name]
                pending_prep.setdefault(q, []).append(ins.name)
                if jq >= PREP_DEPTH and q in queue_sems:
                    sid, snm = queue_sems[q]
                    _add_wait(ins, sid, snm, 16 * (jq - PREP_DEPTH + 1))
            elif tn == "InstTriggerDma":
                if cc_pending and cc_sem is not None:
                    _add_wait(ins, cc_sem[0], cc_sem[1], cc_count)
                    cc_pending = False
                k = getattr(ins, "_count", None)
                lst = pending_prep.get(ins.queue_num, [])
                pns = lst[:k] if k else lst
                pending_prep[ins.queue_num] = lst[len(pns) :]
                if not pns or pe_sem is None:
                    continue
                tgt = 0
                for pn in pns:
                    nm = prep_info[pn][2]
                    prev = prev_inst.get(nm) if nm else None
                    if prev:
                        tgt = max(tgt, last_reader_n.get(prev, 0))
                if tgt > 0:
                    sid, snm = pe_sem
                    _add_wait(ins, sid, snm, tgt)


def _split_sync_waits(nc, mybir, max_waits=1):
    """This walrus build rejects instructions with more than `max_waits` sync
    waits; hoist excess waits onto injected same-engine InstNoOps."""
    n_split = 0
    for fn in nc.m.functions:
        for bb in fn.blocks:
            out = []
            changed = False
            for ins in bb.instructions:
                si = ins.sync_info
                if si is not None and si.on_wait and len(si.on_wait) > max_waits:
                    waits = list(si.on_wait)
                    excess = waits[:-max_waits]
                    for i in range(0, len(excess), max_waits):
                        nop = mybir.InstNoOp(
                            name=nc.get_next_instruction_name(),
                            sync_info=mybir.SyncInfo(
                                on_wait=excess[i : i + max_waits], on_update=[]
                            ),
                            bass_nofuse=True,
                            engine=ins.engine,
                        )
                        out.append(nop)
                        n_split += 1
                    si.on_wait = waits[-max_waits:]
                    ins.sync_info = si
                    changed = True
                out.append(ins)
            if changed:
                bb.instructions = out
    return n_split


# ----------------------------------------------------------------------------
# Entry point
# ----------------------------------------------------------------------------
def kernel(x, edge_index, W1, b1, W2, b2):
    global LAST_RESULTS
    from concourse.bass_utils import run_bass_kernel_spmd

    x = np.asarray(x)
    W1a = np.asarray(W1)
    b1a = np.asarray(b1)
    W2a = np.asarray(W2)
    b2a = np.asarray(b2)

    key = hash(np.asarray(edge_index)[:, :: E // 997].tobytes())
    if key not in _CACHE:
        plan = _plan(edge_index)
        nc = _build(
            plan["T"],
            plan["CA"],
            plan["CB"],
            plan["groups"],
            plan["tot_chunks"],
            plan["maxc_call"],
        )
        _CACHE[key] = (plan, nc)
    plan, nc = _CACHE[key]

    T = plan["T"]
    SLOTS = plan["SLOTS"]
    NT = NCORES * T

    # xT in table order, tile-major: [NT, 128 infeat, 128 nodes]
    xT = np.zeros((NT, P, P), dtype=np.float16)
    gpos = plan["pos_of"]
    xTflat = np.zeros((P, NCORES * SLOTS), dtype=np.float16)
    xTflat[:, gpos] = x.astype(np.float16).T
    xT[:] = xTflat.reshape(P, NT, P).transpose(1, 0, 2)

    in_common = {
        "xT": xT,
        "W1": W1a.astype(np.float16),
        "W2": W2a.astype(np.float16),
        "b1bc": np.broadcast_to(b1a.astype(np.float32), (P, HID)).copy(),
        "b2bc": np.broadcast_to(b2a.astype(np.float32), (P, OUT_CH)).copy(),
        "ident": np.eye(P, dtype=np.float16),
        "dinv_all": plan["dinv_all"],
        "iotaC": plan["iotaC"],
    }
    in_maps = []
    for c in range(NCORES):
        m = dict(in_common)
        m["colidx"] = plan["colidx_cores"][c]
        m["dinv_own"] = plan["dinv_own_cores"][c]
        m["idx"] = plan["idx_cores"][c]
        in_maps.append(m)

    res = run_bass_kernel_spmd(nc, in_maps, core_ids=list(range(NCORES)))
    LAST_RESULTS = res

    out = np.empty((N, OUT_CH), dtype=np.float32)
    core_of = plan["core_of"]
    slot_of = plan["slot_of"]
    for c in range(NCORES):
        sel = core_of == c
        out[sel] = res.results[c]["out"][slot_of[sel]]
    return out


# revision 35
# speedup vs baseline: 2.5101x; 1.8588x over previous
"""GCN 2-layer encoder on 8 TRN2 NeuronCores (Bass/Tile).

Math (PyG GCNConv, symmetric normalization, self-loops, deg from dst):
    out1 = relu(Dh @ A @ Dh @ (x @ W1) + b1),  Dh = diag(deg^-1/2)
    out  = Dh @ A @ Dh @ (relu1 @ W2) + b2

Factorization used here (per layer):
    table = Dh @ (feat @ W)          # per-node rows, built on device
    agg[d] = sum_{e: src->d} table[src]   (self loops included as edges)
    out[d] = dinv[d] * agg[d] + b

Sharding: nodes are assigned to 8 cores (balanced by in-degree); each core
aggregates its own dst nodes. Per dst tile (128 nodes), in-edges are packed
densely into chunks of 128 lanes; gathered message chunks [128 lanes, F]
are multiplied on the PE by a per-chunk multi-hot sigma (lane -> dst col)
accumulating in PSUM. Sigma matrices are built on-device by the Vector
engine (iota == colidx compare) from compact per-lane column indices, so
lanes need no static lane->node binding and padding is just the final
partial chunk per (tile, side): ~5% vs ~56% for per-tile sigma.

Messages are fetched with SWDGE dma_gather in prepare_only mode + explicit
trigger_dma: desc-gen (~0.7us/call) is decoupled from the DMA transfer
(~5-8us/call), which otherwise blocks the GpSimd engine. Post-compile
surgery (_fix_swdge_prep_sems) wires the descriptor completion sems to the
DMASW lane sems Tile's consumers actually wait on, throttles to one
in-flight call per lane, and restores the dropped write-after-read hazard
(trigger vs. previous staging-slot readers).

Since gather indices are int16, the node table is split in two blocks
(cores 0-3 / cores 4-7); each (tile-group, block) run is a separate call.

Layer-1 tables are built replicated on every core; the layer-2 table is
built sharded and exchanged with one AllGather.
"""

import sys
import types

sys.path.insert(0, "/opt/trn_rl_repo")

import numpy as np

# Register the NTFF profile hook the container's antenv stub lacks, so
# BASS_TRACE=1 profiling works under axon (harmless otherwise).
if "antenv.axon_hooks" not in sys.modules:
    try:
        from trn_agent_boot.trn_boot import _ntff_profile_via_ctypes

        _hook = _ntff_profile_via_ctypes("/opt/axon/libaxon_pjrt.so")
    except Exception:
        _hook = None
    _m = types.ModuleType("antenv.axon_hooks")
    _m.get_axon_ntff_profile_hook = lambda: _hook
    sys.modules["antenv.axon_hooks"] = _m

N = 50000
E = 800000
IN_CH = 128
HID = 128
OUT_CH = 64
NCORES = 8
P = 128
GSZ = 4  # tiles per gather call group
CALL_CAP = 8  # max chunks (x128 idxs) per dma_gather call (16KB/engine packet)
SWDGE_QUEUES = 4
BB = 4  # phase-1 DMA batching (tiles per dma_start)

_CACHE = {}
LAST_RESULTS = None


# ----------------------------------------------------------------------------
# Host-side planning
# ----------------------------------------------------------------------------
def _plan(edge_index):
    src = np.asarray(edge_index[0], dtype=np.int64)
    dst = np.asarray(edge_index[1], dtype=np.int64)
    loops = np.arange(N, dtype=np.int64)
    src_all = np.concatenate([src, loops])
    dst_all = np.concatenate([dst, loops])
    deg = np.bincount(dst_all, minlength=N)
    dinv = (1.0 / np.sqrt(deg.astype(np.float64))).astype(np.float32)

    # node -> core: snake over degree-sorted nodes (balances sum(deg))
    order = np.argsort(-deg, kind="stable")
    snake = np.tile(
        np.concatenate([np.arange(NCORES), np.arange(NCORES - 1, -1, -1)]),
        N // (2 * NCORES) + 1,
    )[:N]
    core_of = np.empty(N, dtype=np.int64)
    core_of[order] = snake

    isA = core_of[src_all] < (NCORES // 2)
    a_cnt = np.bincount(dst_all[isA], minlength=N)
    b_cnt = np.bincount(dst_all[~isA], minlength=N)

    # node -> (tile, col): per core, snake over degree-sorted nodes across
    # provisional tiles (balances per-tile edge sums), tiles then sorted by
    # chunk need desc (aligns profiles across cores) and renumbered.
    tile_of = np.full(N, -1, dtype=np.int64)
    col_of = np.full(N, -1, dtype=np.int64)
    ntiles_max = 0
    prov = []
    for c in range(NCORES):
        nodes = np.where(core_of == c)[0]
        nn = len(nodes)
        ntiles = -(-nn // P)
        ntiles_max = max(ntiles_max, ntiles)
        o2 = np.argsort(-(a_cnt[nodes] + b_cnt[nodes]), kind="stable")
        nds = nodes[o2]
        sn = np.tile(
            np.concatenate([np.arange(ntiles), np.arange(ntiles - 1, -1, -1)]),
            nn // (2 * ntiles) + 1,
        )[:nn]
        prov.append([nds[sn == t] for t in range(ntiles)])

    T = ntiles_max
    ca_t = np.zeros((NCORES, T), dtype=np.int64)
    cb_t = np.zeros((NCORES, T), dtype=np.int64)
    for c in range(NCORES):
        for t, nds in enumerate(prov[c]):
            ca_t[c, t] = -(-int(a_cnt[nds].sum()) // P)
            cb_t[c, t] = -(-int(b_cnt[nds].sum()) // P)
    CA = np.zeros(T, dtype=np.int64)
    CB = np.zeros(T, dtype=np.int64)
    for c in range(NCORES):
        perm = sorted(
            range(len(prov[c])), key=lambda t: -(ca_t[c, t] + cb_t[c, t])
        )
        for p_, t in enumerate(perm):
            nds = prov[c][t]
            tile_of[nds] = p_
            col_of[nds] = np.arange(len(nds))
            CA[p_] = max(CA[p_], ca_t[c, t])
            CB[p_] = max(CB[p_], cb_t[c, t])
    CA[(CA + CB) == 0] = 1

    SLOTS = T * P
    HALFROWS = (NCORES // 2) * SLOTS
    assert HALFROWS <= 32768, HALFROWS
    slot_of = tile_of * P + col_of
    pos_of = core_of * SLOTS + slot_of

    # pad rows: any unoccupied slot is a zero row in both tables (zero x,
    # dinv=0). Find one in core 3 (A half) and core 7 (B half).
    def free_slot(c):
        occ = np.zeros(SLOTS, dtype=bool)
        occ[slot_of[core_of == c]] = True
        fr = np.where(~occ)[0]
        assert len(fr) > 0
        return int(fr[-1])

    PAD_A = (NCORES // 2 - 1) * SLOTS + free_slot(NCORES // 2 - 1)
    PAD_B = (NCORES // 2 - 1) * SLOTS + free_slot(NCORES - 1)

    ecore = core_of[dst_all]
    etile = tile_of[dst_all]
    eside = (~isA).astype(np.int64)
    esrcpos = pos_of[src_all]
    ecol = col_of[dst_all]

    G = -(-T // GSZ)
    groups = [list(range(g * GSZ, min((g + 1) * GSZ, T))) for g in range(G)]
    tot_chunks = int(np.sum(CA) + np.sum(CB))
    maxc_call = 0
    for g in groups:
        ca_g = int(sum(CA[p_] for p_ in g))
        cb_g = int(sum(CB[p_] for p_ in g))
        maxc_call = max(maxc_call, ca_g, cb_g)

    ekey = np.lexsort((esrcpos, etile, eside, ecore))
    es_core = ecore[ekey]
    es_side = eside[ekey]
    es_tile = etile[ekey]
    es_srcpos = esrcpos[ekey]
    es_col = ecol[ekey]
    keyv = (es_core * 2 + es_side) * T + es_tile
    uniq, starts = np.unique(keyv, return_index=True)
    ends = np.append(starts[1:], len(keyv))
    bnd = {int(u): (int(s0), int(e0)) for u, s0, e0 in zip(uniq, starts, ends)}

    idx_cores = []
    colidx_cores = []
    dinv_own_cores = []
    for c in range(NCORES):
        flat_idx = []
        flat_col = []
        dvo = np.zeros((P, T), dtype=np.float32)
        nds_c = np.where(core_of == c)[0]
        dvo[col_of[nds_c], tile_of[nds_c]] = dinv[nds_c]

        def emit(side, t, nchunks):
            k = (c * 2 + side) * T + t
            s0, e0 = bnd.get(k, (0, 0))
            sp = es_srcpos[s0:e0]
            cl = es_col[s0:e0]
            if side == 1:
                sp = sp - HALFROWS
            n_ = e0 - s0
            want = nchunks * P
            ii = np.full(want, PAD_A if side == 0 else PAD_B, np.int64)
            cc = np.full(want, P - 1, np.int64)
            ii[:n_] = sp
            cc[:n_] = cl
            flat_idx.append(ii)
            flat_col.append(cc)

        for g in groups:
            for p_ in g:
                emit(0, p_, int(CA[p_]))
            for p_ in g:
                emit(1, p_, int(CB[p_]))
        fi = np.concatenate(flat_idx)
        fc = np.concatenate(flat_col)
        assert fi.size == tot_chunks * P
        assert fi.min() >= 0 and fi.max() < HALFROWS
        wrapped = fi.astype(np.int16).reshape(-1, 16).T.copy()
        idx_cores.append(np.tile(wrapped, (8, 1)))
        colidx_cores.append(fc.reshape(tot_chunks, P).T.astype(np.float16).copy())
        dinv_own_cores.append(dvo)

    dinv_all = np.zeros((P, NCORES * T), dtype=np.float32)
    for c in range(NCORES):
        dinv_all[:, c * T : (c + 1) * T] = dinv_own_cores[c]

    iotaC = np.tile(
        np.arange(P, dtype=np.float16)[None, :], (P, maxc_call)
    ).reshape(P, maxc_call * P)

    return dict(
        T=T,
        SLOTS=SLOTS,
        CA=CA,
        CB=CB,
        groups=groups,
        tot_chunks=tot_chunks,
        maxc_call=maxc_call,
        core_of=core_of,
        slot_of=slot_of,
        pos_of=pos_of,
        dinv=dinv,
        idx_cores=idx_cores,
        colidx_cores=colidx_cores,
        dinv_own_cores=dinv_own_cores,
        dinv_all=dinv_all,
        iotaC=iotaC,
    )


# ----------------------------------------------------------------------------
# Device kernel
# ----------------------------------------------------------------------------
def _build(
    T,
    CA,
    CB,
    groups,
    tot_chunks,
    maxc_call,
    use_collective=True,
    detect_races=True,
):
    import concourse.bass as bass
    import concourse.mybir as mybir
    import concourse.tile as tile
    from concourse import bacc

    f16 = mybir.dt.float16
    f32 = mybir.dt.float32
    i16 = mybir.dt.int16
    SLOTS = T * P
    ROWS = NCORES * SLOTS
    HALFROWS = ROWS // 2
    NT = NCORES * T

    nc = bacc.Bacc(
        "TRN2",
        target_bir_lowering=False,
        num_devices=NCORES,
        num_swdge_queues=SWDGE_QUEUES,
        detect_race_conditions=detect_races,
    )
    qn = [0]

    def _next_q():
        qn[0] = (qn[0] + 1) % SWDGE_QUEUES
        return qn[0]

    # R rotating completion sems per queue: consumer waits target call's
    # rotation sem, so a premature unblock needs >= R calls of inter-engine
    # skew on one queue instead of 1.
    dma_sems = [
        [nc.alloc_semaphore(f"swdge_dma_q{i}r{r}") for r in range(SEM_ROT)]
        for i in range(SWDGE_QUEUES)
    ]
    q_calls = [0] * SWDGE_QUEUES

    def _prep(out_ap, in_ap, idx_ap, n_idx, q):
        jq = q_calls[q]
        q_calls[q] += 1
        nc.gpsimd.dma_gather(
            out_ap,
            in_ap,
            idx_ap,
            n_idx,
            n_idx,
            P,
            prepare_only=True,
            sem=dma_sems[q][jq % SEM_ROT],
            queue_num=q,
        )

    def _fire(q):
        # One trigger per (group, side): the trigger blocks the GpSimd
        # engine ~9us on HW regardless of how many calls it fires, so
        # batch all of a side's calls onto one queue and fire once.
        nc.gpsimd.trigger_dma(count=None, queue_num=q)

    xT_in = nc.dram_tensor("xT", [NT, P, P], f16, kind="ExternalInput")
    w1_in = nc.dram_tensor("W1", [IN_CH, HID], f16, kind="ExternalInput")
    w2_in = nc.dram_tensor("W2", [HID, OUT_CH], f16, kind="ExternalInput")
    b1_in = nc.dram_tensor("b1bc", [P, HID], f32, kind="ExternalInput")
    b2_in = nc.dram_tensor("b2bc", [P, OUT_CH], f32, kind="ExternalInput")
    id_in = nc.dram_tensor("ident", [P, P], f16, kind="ExternalInput")
    col_in = nc.dram_tensor("colidx", [P, tot_chunks], f16, kind="ExternalInput")
    iota_in = nc.dram_tensor("iotaC", [P, maxc_call * P], f16, kind="ExternalInput")
    da_in = nc.dram_tensor("dinv_all", [P, NT], f32, kind="ExternalInput")
    do_in = nc.dram_tensor("dinv_own", [P, T], f32, kind="ExternalInput")
    idx_in = nc.dram_tensor("idx", [P, tot_chunks * 8], i16, kind="ExternalInput")
    out_ext = nc.dram_tensor("out", [SLOTS, OUT_CH], f32, kind="ExternalOutput")

    with tile.TileContext(nc) as tc:
        with (
            tc.tile_pool(name="const", bufs=1) as cpool,
            tc.tile_pool(name="xt", bufs=3) as xtpool,
            tc.tile_pool(name="sig", bufs=3) as sigpool,
            tc.tile_pool(name="stg", bufs=3) as stgpool,
            tc.tile_pool(name="drain", bufs=3) as dpool,
            tc.tile_pool(name="psb", bufs=2, space="PSUM") as ps_build,
            tc.tile_pool(name="psa", bufs=2, space="PSUM") as ps_agg,
            tc.tile_pool(name="pst", bufs=2, space="PSUM") as ps_tr,
            tc.tile_pool(name="psm", bufs=2, space="PSUM") as ps_mm2,
            tc.tile_pool(name="dram", bufs=1, space="DRAM") as dram,
        ):
            # ---- constants into SBUF ----
            w1_sb = cpool.tile([IN_CH, HID], f16)
            nc.sync.dma_start(out=w1_sb[:], in_=w1_in[:])
            w2_sb = cpool.tile([HID, OUT_CH], f16)
            nc.sync.dma_start(out=w2_sb[:], in_=w2_in[:])
            b1_sb = cpool.tile([P, HID], f32)
            nc.sync.dma_start(out=b1_sb[:], in_=b1_in[:])
            b2_sb = cpool.tile([P, OUT_CH], f32)
            nc.sync.dma_start(out=b2_sb[:], in_=b2_in[:])
            id_sb = cpool.tile([P, P], f16)
            nc.sync.dma_start(out=id_sb[:], in_=id_in[:])
            col_sb = cpool.tile([P, tot_chunks], f16)
            nc.sync.dma_start(out=col_sb[:], in_=col_in[:])
            iota_sb = cpool.tile([P, maxc_call * P], f16)
            nc.sync.dma_start(out=iota_sb[:], in_=iota_in[:])
            da_sb = cpool.tile([P, NT], f32)
            nc.sync.dma_start(out=da_sb[:], in_=da_in[:])
            do_sb = cpool.tile([P, T], f32)
            nc.sync.dma_start(out=do_sb[:], in_=do_in[:])
            idx_sb = cpool.tile([P, tot_chunks * 8], i16)
            nc.sync.dma_start(out=idx_sb[:], in_=idx_in[:])

            table1 = dram.tile([ROWS, HID], f16)
            shard2 = dram.tile([SLOTS, P], f16)
            table2 = dram.tile(
                [ROWS, P], f16, addr_space="Shared" if use_collective else "Local"
            )

            # ---- phase 1: table1 = dinv * (x @ W1), full, replicated ----
            for j0 in range(0, NT, BB):
                nb = min(BB, NT - j0)
                xt_t = xtpool.tile([P, nb * P], f16, tag="xt")
                nc.sync.dma_start(
                    out=xt_t[:].rearrange("p (t c) -> p t c", t=nb),
                    in_=xT_in[j0 : j0 + nb].rearrange("t p c -> p t c"),
                )
                h1t = xtpool.tile([P, nb * HID], f16, tag="h1t")
                for k in range(nb):
                    j = j0 + k
                    bps = ps_build.tile([P, HID], f32, tag="build")
                    nc.tensor.matmul(
                        bps[:],
                        lhsT=xt_t[:, k * P : (k + 1) * P],
                        rhs=w1_sb[:],
                        start=True,
                        stop=True,
                    )
                    if j % 2 == 0:
                        nc.scalar.activation(
                            h1t[:, k * HID : (k + 1) * HID],
                            bps[:],
                            mybir.ActivationFunctionType.Copy,
                            scale=da_sb[:, j : j + 1],
                        )
                    else:
                        nc.vector.tensor_scalar_mul(
                            h1t[:, k * HID : (k + 1) * HID],
                            bps[:],
                            da_sb[:, j : j + 1],
                        )
                nc.sync.dma_start(
                    out=table1[j0 * P : (j0 + nb) * P, :].rearrange(
                        "(t p) f -> p t f", t=nb
                    ),
                    in_=h1t[:].rearrange("p (t f) -> p t f", t=nb),
                )

            # ---- per-layer aggregation ----
            def aggregate(layer):
                tab = table1 if layer == 0 else table2
                nfeat = HID if layer == 0 else OUT_CH
                coff = 0
                for g in groups:
                    ca_g = int(sum(int(CA[p_]) for p_ in g))
                    cb_g = int(sum(int(CB[p_]) for p_ in g))
                    stA = stB = sgA = sgB = None
                    if ca_g:
                        stA = stgpool.tile([P, maxc_call, P], f16, tag="stgA")
                        used = set()
                        for s_ in range(0, ca_g, CALL_CAP):
                            n_ = min(CALL_CAP, ca_g - s_)
                            q = _next_q()
                            used.add(q)
                            _prep(
                                stA[:, s_ : s_ + n_, :],
                                tab[0:HALFROWS, :],
                                idx_sb[:, (coff + s_) * 8 : (coff + s_ + n_) * 8],
                                n_ * P,
                                q,
                            )
                        for q in sorted(used):
                            _fire(q)
                        sgA = sigpool.tile([P, maxc_call * P], f16, tag="sgA")
                        nc.vector.tensor_tensor(
                            sgA[:, : ca_g * P].rearrange(
                                "p (k c) -> p k c", k=ca_g
                            ),
                            iota_sb[:, : ca_g * P].rearrange(
                                "p (k c) -> p k c", k=ca_g
                            ),
                            col_sb[:, coff : coff + ca_g]
                            .unsqueeze(-1)
                            .broadcast_to([P, ca_g, P]),
                            mybir.AluOpType.is_equal,
                        )
                    if cb_g:
                        stB = stgpool.tile([P, maxc_call, P], f16, tag="stgB")
                        used = set()
                        for s_ in range(0, cb_g, CALL_CAP):
                            n_ = min(CALL_CAP, cb_g - s_)
                            q = _next_q()
                            used.add(q)
                            _prep(
                                stB[:, s_ : s_ + n_, :],
                                tab[HALFROWS:ROWS, :],
                                idx_sb[
                                    :,
                                    (coff + ca_g + s_) * 8 : (coff + ca_g + s_ + n_)
                                    * 8,
                                ],
                                n_ * P,
                                q,
                            )
                        for q in sorted(used):
                            _fire(q)
                        sgB = sigpool.tile([P, maxc_call * P], f16, tag="sgB")
                        nc.vector.tensor_tensor(
                            sgB[:, : cb_g * P].rearrange(
                                "p (k c) -> p k c", k=cb_g
                            ),
                            iota_sb[:, : cb_g * P].rearrange(
                                "p (k c) -> p k c", k=cb_g
                            ),
                            col_sb[:, coff + ca_g : coff + ca_g + cb_g]
                            .unsqueeze(-1)
                            .broadcast_to([P, cb_g, P]),
                            mybir.AluOpType.is_equal,
                        )
                    a_off = 0
                    b_off = 0
                    for p_ in g:
                        aps = ps_agg.tile([P, nfeat], f32, tag="agg")
                        ntot = int(CA[p_]) + int(CB[p_])
                        k = 0
                        for ci in range(int(CA[p_])):
                            cc = a_off + ci
                            nc.tensor.matmul(
                                aps[:],
                                lhsT=sgA[:, cc * P : (cc + 1) * P],
                                rhs=stA[:, cc, 0:nfeat],
                                start=(k == 0),
                                stop=(k == ntot - 1),
                            )
                            k += 1
                        for ci in range(int(CB[p_])):
                            cc = b_off + ci
                            nc.tensor.matmul(
                                aps[:],
                                lhsT=sgB[:, cc * P : (cc + 1) * P],
                                rhs=stB[:, cc, 0:nfeat],
                                start=(k == 0),
                                stop=(k == ntot - 1),
                            )
                            k += 1
                        a_off += int(CA[p_])
                        b_off += int(CB[p_])
                        drain(layer, p_, aps)
                    coff += ca_g + cb_g

            def drain(layer, p_, aps):
                dv = do_sb[:, p_ : p_ + 1]
                if layer == 0:
                    # r1 = dinv*agg + b1 ; r3 = relu(r1)*dinv (fp16)
                    r1 = dpool.tile([P, HID], f32, tag="r1")
                    nc.scalar.activation(
                        r1[:], aps[:], mybir.ActivationFunctionType.Copy, scale=dv
                    )
                    nc.vector.tensor_add(r1[:], r1[:], b1_sb[:])
                    r3 = dpool.tile([P, HID], f16, tag="r3")
                    nc.vector.tensor_scalar(
                        r3[:], r1[:], 0.0, dv, mybir.AluOpType.max, mybir.AluOpType.mult
                    )
                    psT = ps_tr.tile([P, P], f16, tag="tr")
                    nc.tensor.transpose(psT[:], r3[:], id_sb[:])
                    rT = dpool.tile([P, P], f16, tag="rT")
                    nc.vector.tensor_copy(rT[:], psT[:])
                    ps2 = ps_mm2.tile([P, OUT_CH], f32, tag="mm2")
                    nc.tensor.matmul(
                        ps2[:], lhsT=rT[:], rhs=w2_sb[:], start=True, stop=True
                    )
                    t2 = dpool.tile([P, P], f16, tag="t2")
                    nc.scalar.activation(
                        t2[:, 0:OUT_CH], ps2[:], mybir.ActivationFunctionType.Copy
                    )
                    nc.vector.memset(t2[:, OUT_CH:P], 0.0)
                    nc.sync.dma_start(
                        out=shard2[p_ * P : (p_ + 1) * P, :], in_=t2[:]
                    )
                else:
                    o1 = dpool.tile([P, OUT_CH], f32, tag="o1")
                    nc.scalar.activation(
                        o1[:], aps[:], mybir.ActivationFunctionType.Copy, scale=dv
                    )
                    nc.vector.tensor_add(o1[:], o1[:], b2_sb[:])
                    nc.sync.dma_start(
                        out=out_ext[p_ * P : (p_ + 1) * P, :], in_=o1[:]
                    )

            aggregate(0)

            if use_collective:
                nc.gpsimd.collective_compute(
                    "AllGather",
                    mybir.AluOpType.bypass,
                    replica_groups=[list(range(NCORES))],
                    ins=[shard2.opt()],
                    outs=[table2.opt()],
                )
            else:
                for c_ in range(NCORES):
                    nc.sync.dma_start(
                        out=table2[c_ * SLOTS : (c_ + 1) * SLOTS, :], in_=shard2[:]
                    )

            aggregate(1)

    nc.compile()  # bacc passes: library loads, register allocation, DCE
    _fix_swdge_prep_sems(nc, mybir)
    _split_sync_waits(nc, mybir, max_waits=1)
    return nc


PREP_DEPTH = 10  # max in-flight gather calls per SWDGE queue
STG_BUFS = 3  # staging pool depth (groups in flight); must match tile_pool
SEM_ROT = 4  # rotating DMA-completion sems per queue


def _fix_swdge_prep_sems(nc, mybir):
    """Post-compile surgery for the gen_mode==1 SWDGE prep/trigger path.

    Tile treats prepare_only SWDGE completion as user-managed: it
    discharges the DMASW lane ticks with unconditional IncSwdgeSem
    pre-bumps, so the lane-sem waits it emits on consumers are vacuous.
    The author must enforce data readiness with the sem= semaphores
    (one per queue here, descriptors bump +16 per call). Enforce:

    1. Data RAW: the first matmul reading each staging-tile instance
       waits on every covering gather call: sem_q >= 16*(call# in q + 1).
    2. WAR: the trigger that fires a DMA overwriting a staging slot waits
       on PE engine sem >= (last matmul reading the slot's previous
       instance; with STG_BUFS pool bufs that is the same-tag instance
       STG_BUFS allocations back).
    3. Ring/throttle: prep #j on queue q waits sem_q >= 16*(j-D+1),
       capping in-flight calls per queue at D=PREP_DEPTH.
    """
    import re

    queue_sems = {}  # (q, r) -> (id, name)
    pe_sem = None
    for fn in nc.m.functions:
        for bb in fn.blocks:
            for ins in bb.instructions:
                si = ins.sync_info
                if not si:
                    continue
                for u in si.on_update or []:
                    nm = u.ant_name or ""
                    if nm.startswith("swdge_dma_q"):
                        qs, rs = nm[11:].split("r")
                        queue_sems[(int(qs), int(rs))] = (u.id, nm)
                for w in si.on_wait or []:
                    nm = w.ant_name or ""
                    if nm.startswith("PE_") and pe_sem is None:
                        pe_sem = (w.id, nm)

    pat = re.compile(r"\b(st[AB])_(\d+)\b")

    def _stg_name(ap):
        m = pat.search(str(ap))
        return m.group(0) if m else None

    streams = []
    for fn in nc.m.functions:
        for bb in fn.blocks:
            streams.append(bb.instructions)

    # pass 1: per-queue call indices per prep; staging instances: creation
    # order (per tag), covering calls, first/last matmul readers
    inst_order = {"stA": [], "stB": []}
    seen = set()
    first_reader = {}
    last_reader_n = {}
    inst_calls = {}
    prep_info = {}
    q_count = {}
    pe_n = 0
    for insts in streams:
        for ins in insts:
            tn = type(ins).__name__
            if tn == "InstMatmult":
                pe_n += 1
                for ap in ins.ins or []:
                    nm = _stg_name(ap)
                    if nm:
                        if nm not in first_reader:
                            first_reader[nm] = ins
                        last_reader_n[nm] = pe_n
            elif tn == "InstDMAGatherAnt" and getattr(ins, "gen_mode", 0) == 1:
                q = ins.queue_num
                jq = q_count.get(q, 0)
                q_count[q] = jq + 1
                nm = _stg_name(ins.outs[0])
                prep_info[ins.name] = (q, jq, nm)
                if nm:
                    inst_calls.setdefault(nm, []).append((q, jq))
                    if nm not in seen:
                        seen.add(nm)
                        inst_order[nm[:3]].append(nm)
    prev_inst = {}
    for tag, lst in inst_order.items():
        for i, nm in enumerate(lst):
            if i >= STG_BUFS:
                prev_inst[nm] = lst[i - STG_BUFS]

    def _add_wait(ins, sid, snm, val):
        si = ins.sync_info
        if si is None:
            si = mybir.SyncInfo(on_wait=[], on_update=[])
        si.on_wait = list(si.on_wait or []) + [
            mybir.SyncWait(
                sync_type="semaphore",
                id=sid,
                ant_name=snm,
                wait_mode="sem-ge-imm",
                wait_value=val,
                wait_reg=None,
            )
        ]
        ins.sync_info = si

    # 1. data RAW waits on first readers (per (queue, rotation) max target)
    for nm, rd in first_reader.items():
        per_qr = {}
        for q, jq in inst_calls.get(nm, []):
            k = (q, jq % SEM_ROT)
            per_qr[k] = max(per_qr.get(k, -1), jq // SEM_ROT)
        for k, t in sorted(per_qr.items()):
            if k in queue_sems:
                sid, snm = queue_sems[k]
                _add_wait(rd, sid, snm, 16 * (t + 1))

    # 2 + 3. WAR waits on triggers, throttle on preps; also gate the first
    # trigger after each collective on its completion (the gather source
    # table2 is written by the async AllGather).
    cc_sem = None
    for insts in streams:
        for ins in insts:
            si = ins.sync_info
            if not si:
                continue
            for w in si.on_wait or []:
                if (w.ant_name or "").startswith("Collectives"):
                    cc_sem = (w.id, w.ant_name)
    pending_prep = {}
    cc_count = 0
    cc_pending = set()
    for insts in streams:
        for ins in insts:
            tn = type(ins).__name__
            if tn == "InstCollectiveCompute":
                cc_count += 1
                cc_pending = set(range(SWDGE_QUEUES))
            elif tn == "InstDMAGatherAnt" and getattr(ins, "gen_mode", 0) == 1:
                q, jq, nm = prep_info[ins.name]
                pending_prep.setdefault(q, []).append(ins.name)
                if jq >= PREP_DEPTH:
                    jt = jq - PREP_DEPTH
                    k = (q, jt % SEM_ROT)
                    if k in queue_sems:
                        sid, snm = queue_sems[k]
                        _add_wait(ins, sid, snm, 16 * (jt // SEM_ROT + 1))
            elif tn == "InstTriggerDma":
                if ins.queue_num in cc_pending and cc_sem is not None:
                    _add_wait(ins, cc_sem[0], cc_sem[1], cc_count)
                    cc_pending.discard(ins.queue_num)
                k = getattr(ins, "_count", None)
                lst = pending_prep.get(ins.queue_num, [])
                pns = lst[:k] if k else lst
                pending_prep[ins.queue_num] = lst[len(pns) :]
                if not pns or pe_sem is None:
                    continue
                tgt = 0
                for pn in pns:
                    nm = prep_info[pn][2]
                    prev = prev_inst.get(nm) if nm else None
                    if prev:
                        tgt = max(tgt, last_reader_n.get(prev, 0))
                if tgt > 0:
                    sid, snm = pe_sem
                    _add_wait(ins, sid, snm, tgt)


def _split_sync_waits(nc, mybir, max_waits=1):
    """This walrus build rejects instructions with more than `max_waits` sync
    waits; hoist excess waits onto injected same-engine InstNoOps."""
    n_split = 0
    for fn in nc.m.functions:
        for bb in fn.blocks:
            out = []
            changed = False
            for ins in bb.instructions:
                si = ins.sync_info
                if si is not None and si.on_wait and len(si.on_wait) > max_waits:
                    waits = list(si.on_wait)
                    excess = waits[:-max_waits]
                    for i in range(0, len(excess), max_waits):
                        nop = mybir.InstNoOp(
                            name=nc.get_next_instruction_name(),
                            sync_info=mybir.SyncInfo(
                                on_wait=excess[i : i + max_waits], on_update=[]
                            ),
                            bass_nofuse=True,
                            engine=ins.engine,
                        )
                        out.append(nop)
                        n_split += 1
                    si.on_wait = waits[-max_waits:]
                    ins.sync_info = si
                    changed = True
                out.append(ins)
            if changed:
                bb.instructions = out
    return n_split


# ----------------------------------------------------------------------------
# Entry point
# ----------------------------------------------------------------------------
def kernel(x, edge_index, W1, b1, W2, b2):
    global LAST_RESULTS
    from concourse.bass_utils import run_bass_kernel_spmd

    x = np.asarray(x)
    W1a = np.asarray(W1)
    b1a = np.asarray(b1)
    W2a = np.asarray(W2)
    b2a = np.asarray(b2)

    key = hash(np.asarray(edge_index)[:, :: E // 997].tobytes())
    if key not in _CACHE:
        plan = _plan(edge_index)
        nc = _build(
            plan["T"],
            plan["CA"],
            plan["CB"],
            plan["groups"],
            plan["tot_chunks"],
            plan["maxc_call"],
        )
        _CACHE[key] = (plan, nc)
    plan, nc = _CACHE[key]

    T = plan["T"]
    SLOTS = plan["SLOTS"]
    NT = NCORES * T

    # xT in table order, tile-major: [NT, 128 infeat, 128 nodes]
    xT = np.zeros((NT, P, P), dtype=np.float16)
    gpos = plan["pos_of"]
    xTflat = np.zeros((P, NCORES * SLOTS), dtype=np.float16)
    xTflat[:, gpos] = x.astype(np.float16).T
    xT[:] = xTflat.reshape(P, NT, P).transpose(1, 0, 2)

    in_common = {
        "xT": xT,
        "W1": W1a.astype(np.float16),
        "W2": W2a.astype(np.float16),
        "b1bc": np.broadcast_to(b1a.astype(np.float32), (P, HID)).copy(),
        "b2bc": np.broadcast_to(b2a.astype(np.float32), (P, OUT_CH)).copy(),
        "ident": np.eye(P, dtype=np.float16),
        "dinv_all": plan["dinv_all"],
        "iotaC": plan["iotaC"],
    }
    in_maps = []
    for c in range(NCORES):
        m = dict(in_common)
        m["colidx"] = plan["colidx_cores"][c]
        m["dinv_own"] = plan["dinv_own_cores"][c]
        m["idx"] = plan["idx_cores"][c]
        in_maps.append(m)

    res = run_bass_kernel_spmd(nc, in_maps, core_ids=list(range(NCORES)))
    LAST_RESULTS = res

    out = np.empty((N, OUT_CH), dtype=np.float32)
    core_of = plan["core_of"]
    slot_of = plan["slot_of"]
    for c in range(NCORES):
        sel = core_of == c
        out[sel] = res.results[c]["out"][slot_of[sel]]
    return out
